# revision 1
# baseline (speedup 1.0000x reference)
"""GQA causal-attention prefill kernel for Trainium2, tensor-parallel over 8 NeuronCores.

Reference semantics (see problem): q/k/v projections + RoPE + causal GQA
attention + output projection, fp32, B=2, T=2048, D=4096, 32 q heads,
8 kv heads, head_dim 128.

Sharding: head-parallel. Core c gets q heads [4c, 4c+4), kv head c, and the
matching wo slice; each core computes a full-shape partial output
o_part = attn(heads of c) @ wo_c and the host sums the 8 partials
(the tensor-parallel all-reduce, done at unshard time).

Layout strategy on-core (all matmuls fp32r on the PE):
  - x is passed pre-transposed (xT [D, B*T]) so projections contract D on
    the partition dim:  qT/kT/vT[h] = w[h].T @ xT  -> [H=128, tokens].
  - RoPE applied during PSUM eviction (halves of the H partition dim).
  - scores are computed transposed (sT[s, t] = kT_tile.T @ qT) so the
    expensive softmax reduction over s becomes a matmul-side reduction:
    v is stored natural [s, H] with a ones column appended, so
    out_nat[t, 0:128] = sum_s p[s,t] v[s,:] and out_nat[t, 128] = l[t]
    (the softmax denominator) come out of one accumulation group.
  - softmax skips the max-shift (scores/sqrt(H) ~ N(0,1) here, exp is safe
    in fp32); exp is fused into the PSUM eviction on the scalar engine.
  - causal mask = multiply by a 0/1 wedge mask on the diagonal band blocks.
  - normalization folds into the out_nat eviction (per-partition 1/l).
  - out_nat is PE-transposed so the o-projection contracts (h, H) on the
    partition dim against the natural wo layout.
"""

import os
import sys

sys.path.insert(0, "/opt/trn_rl_repo")

import numpy as np

B = 2
T = 2048
TOK = B * T
D = 4096
NQ = 32
NKV = 8
H = 128
HH = H // 2
THETA = 10000.0
NCORES = 8
NHC = NQ // NCORES          # q heads per core (4)
KPC = D // H                # contraction chunks of 128 over D (32)
TCH = 512                   # token chunk for projections / scores free dim
NTCH = T // TCH             # 4 token chunks per batch
C_SM = 1.0 / np.sqrt(H)     # softmax scale


def _build_bass():
    import concourse.bacc as bacc
    import concourse.mybir as mybir
    import concourse.tile as tile
    from concourse.masks import make_identity

    f32 = mybir.dt.float32
    f32r = mybir.dt.float32r
    Exp = mybir.ActivationFunctionType.Exp

    nc = bacc.Bacc("TRN2", target_bir_lowering=False, debug=False,
                   num_devices=NCORES)

    xT = nc.declare_dram_parameter("xT", [D, TOK], f32, isOutput=False)
    wq = nc.declare_dram_parameter("wq", [NHC, D, H], f32, isOutput=False)
    wk = nc.declare_dram_parameter("wk", [D, H], f32, isOutput=False)
    wv = nc.declare_dram_parameter("wv", [D, H], f32, isOutput=False)
    wo = nc.declare_dram_parameter("wo", [NHC, H, D], f32, isOutput=False)
    # rope tables duplicated across both partition halves: row p and row
    # p+64 hold the same values, so every rope operand pair shares a base.
    cosT = nc.declare_dram_parameter("cosT", [H, TOK], f32, isOutput=False)
    sinT = nc.declare_dram_parameter("sinT", [H, TOK], f32, isOutput=False)
    o_part = nc.declare_dram_parameter("o_part", [TOK, D], f32, isOutput=True)

    with tile.TileContext(nc) as tc:
        from contextlib import ExitStack

        with ExitStack() as top:
            # fp32r-consumed constants need their own tensors: the walrus
            # "rounded to FP32r" producer check is tensor-granular.
            consts = top.enter_context(tc.tile_pool(name="consts", bufs=1))
            identity = consts.tile([H, H], f32)
            make_identity(nc, identity)
            ones_f32 = consts.tile([H, 1], f32, tag="ones32")
            nc.vector.memset(ones_f32, 1.0)
            ones_col = consts.tile([H, 1], f32r, tag="ones")
            nc.vector.tensor_copy(ones_col, ones_f32)
            ones_row_f32 = consts.tile([1, H], f32, tag="onesrow32")
            nc.vector.memset(ones_row_f32, 1.0)
            ones_row = consts.tile([1, H], f32r, tag="onesrow")
            nc.vector.tensor_copy(ones_row, ones_row_f32)
            # 0/1 causal wedge masks for the diagonal band:
            # mask[j][s, t] = 1 iff (t - s - 128*j) >= 0
            masks = []
            for j in range(TCH // H):
                m = consts.tile([H, TCH], f32, tag=f"mask{j}",
                                name=f"mask{j}")
                nc.vector.memset(m, 1.0)
                nc.gpsimd.affine_select(
                    out=m, in_=m,
                    compare_op=mybir.AluOpType.is_ge,
                    fill=0.0,
                    base=-H * j,
                    pattern=[[1, TCH]],
                    channel_multiplier=-1,
                )
                masks.append(m)
            for b in range(B):
                tb = b * T
                with ExitStack() as bstk:
                    act = bstk.enter_context(tc.tile_pool(name="act", bufs=1))
                    # activations for this batch (consumed by phase 2), split
                    # per t-chunk: Tile dependency tracking is tile-granular,
                    # so one big tile would make phase 2's first reads wait on
                    # the LAST chunk's eviction tail.
                    qTs = [act.tile([H, NHC, TCH], f32r, tag=f"qT{i}",
                                    name=f"qT{i}") for i in range(NTCH)]
                    kTs = [act.tile([H, TCH], f32r, tag=f"kT{i}",
                                    name=f"kT{i}") for i in range(NTCH)]
                    # v natural: [s within tile, s-tile-within-chunk, H]
                    vs = [act.tile([H, TCH // H, H], f32r, tag=f"v{i}",
                                   name=f"v{i}") for i in range(NTCH)]

                    # phase 1: projections + rope in ONE x-sweep:
                    # 6 accumulation groups (q0-q3, k, v) in 6 PSUM banks plus
                    # 2 transpose banks. Banks are single-buffered; evictions
                    # are staged out via one ACT copy + one DVE half-swap copy
                    # per bank so each bank frees in well under a microsecond,
                    # and the rope math runs on SBUF staging off the critical
                    # path (DVE muls + GpSimd add/sub).
                    with ExitStack() as ph1:
                        wpool = ph1.enter_context(
                            tc.tile_pool(name="wpool", bufs=1))
                        xpool = ph1.enter_context(
                            tc.tile_pool(name="xpool", bufs=4))
                        rtmp = ph1.enter_context(
                            tc.tile_pool(name="rtmp", bufs=2))
                        pj = ph1.enter_context(
                            tc.tile_pool(name="pj", bufs=1, space="PSUM"))
                        pt = ph1.enter_context(
                            tc.tile_pool(name="pt", bufs=2, space="PSUM"))

                        # per-head wq tiles: deps are tile-granular, so the
                        # first matmul of the batch only waits for head 0's
                        # 2MB instead of the whole 8MB load
                        wq_src = (wq.rearrange("h (c p) m -> p h c m", p=H)
                                  .bitcast(f32r))
                        wqs = []
                        for i in range(NHC):
                            wq_h = wpool.tile([H, KPC, H], f32r, tag=f"wq{i}",
                                              name=f"wq{i}")
                            for c8 in range(4):
                                sl = slice(c8 * 8, (c8 + 1) * 8)
                                nc.sync.dma_start(out=wq_h[:, sl, :],
                                                  in_=wq_src[:, i, sl, :])
                            wqs.append(wq_h)
                        wk_sb = wpool.tile([H, KPC, H], f32r, tag="wk")
                        wk_src = (wk.rearrange("(c p) m -> p c m", p=H)
                                  .bitcast(f32r))
                        wv_sb = wpool.tile([H, KPC, H], f32r, tag="wv")
                        wv_src = (wv.rearrange("(c p) m -> p c m", p=H)
                                  .bitcast(f32r))
                        for c16 in range(2):
                            sl = slice(c16 * 16, (c16 + 1) * 16)
                            nc.sync.dma_start(out=wk_sb[:, sl, :],
                                              in_=wk_src[:, sl, :])
                            nc.sync.dma_start(out=wv_sb[:, sl, :],
                                              in_=wv_src[:, sl, :])
                        cos_sb = wpool.tile([H, T], f32, tag="cos")
                        nc.sync.dma_start(out=cos_sb, in_=cosT[:, tb:tb + T])
                        sin_sb = wpool.tile([H, T], f32, tag="sin")
                        nc.sync.dma_start(out=sin_sb, in_=sinT[:, tb:tb + T])

                        def rope_release(psum):
                            # free the PSUM bank fast: ACT copies the bank
                            # straight out, DVE copies it half-swapped; the
                            # rope math later reads SBUF staging only.
                            # All groups' releases are emitted before any math
                            # so no bank release queues behind rope muls on
                            # DVE (per-proc ticks are globally ordered).
                            direct = rtmp.tile([H, TCH], f32, tag="rdir",
                                               bufs=5, name="direct")
                            swap = rtmp.tile([H, TCH], f32, tag="rswap",
                                             bufs=5, name="swap")
                            nc.scalar.activation(
                                direct, psum,
                                mybir.ActivationFunctionType.Copy)
                            nc.vector.tensor_copy(swap[0:HH, :], psum[HH:H, :])
                            nc.vector.tensor_copy(swap[HH:H, :], psum[0:HH, :])
                            return direct, swap

                        def rope_math(direct, swap, dst_first, dst_second,
                                      cs, sn):
                            # (both-SBUF operand pairs must share a base
                            # partition, hence the swapped staging copy.)
                            # All four muls write plain-f32 temps (f32r cast
                            # writes run ~2.4x slower on DVE); GpSimd combines
                            # the products and does the single f32r write, so
                            # each dst has one writer and DVE never waits on
                            # GpSimd.
                            tmp = rtmp.tile([H, TCH], f32, tag="rt", bufs=2)
                            tmp2 = rtmp.tile([H, TCH], f32, tag="rt2", bufs=2)
                            t1 = tmp[0:HH, :]
                            t2 = tmp[HH:H, :]
                            c1 = tmp2[0:HH, :]
                            c2 = tmp2[HH:H, :]
                            nc.vector.tensor_mul(t1, swap[0:HH, :], sn[0:HH, :])
                            nc.vector.tensor_mul(c1, direct[0:HH, :],
                                                 cs[0:HH, :])
                            nc.gpsimd.tensor_sub(dst_first, c1, t1)
                            nc.vector.tensor_mul(t2, swap[HH:H, :], sn[HH:H, :])
                            nc.vector.tensor_mul(c2, direct[HH:H, :],
                                                 cs[HH:H, :])
                            nc.gpsimd.tensor_add(dst_second, c2, t2)

                        last = KPC - 1
                        for tch in range(NTCH):
                            t0 = tch * TCH
                            g_ps = [pj.tile([H, TCH], f32, tag=f"g{i}",
                                            name=f"g_ps{i}")
                                    for i in range(6)]
                            for k in range(KPC):
                                x_t = xpool.tile([H, TCH], f32r, tag="x")
                                nc.sync.dma_start(
                                    out=x_t,
                                    in_=xT[k * H:(k + 1) * H,
                                           tb + t0:tb + t0 + TCH]
                                    .bitcast(f32r))
                                lhs = [wqs[0][:, k, :], wqs[1][:, k, :],
                                       wqs[2][:, k, :], wqs[3][:, k, :],
                                       wk_sb[:, k, :], wv_sb[:, k, :]]
                                for i in range(6):
                                    nc.tensor.matmul(
                                        g_ps[i], lhs[i], x_t,
                                        start=(k == 0), stop=(k == last),
                                        skip_group_check=True)
                            cs = cos_sb[:, t0:t0 + TCH]
                            sn = sin_sb[:, t0:t0 + TCH]
                            # v first: the transposes are the only PE work in
                            # the eviction tail, so emitting them before the
                            # rope chain keeps the tail off the PE's critical
                            # path at the phase boundary.
                            vt_stage = rtmp.tile([H, TCH], f32,
                                                 tag="vstage", bufs=1)
                            nc.vector.tensor_copy(vt_stage, g_ps[5])
                            for j in range(TCH // H):
                                tp = pt.tile([H, H], f32, tag="vtp")
                                nc.tensor.transpose(
                                    tp, vt_stage[:, j * H:(j + 1) * H],
                                    identity)
                                nc.vector.tensor_copy(vs[tch][:, j, :], tp)
                            # release banks in the order the next chunk's
                            # matmuls need them (q0..q3, k); kT's math runs
                            # first since phase 2 consumes kT earliest.
                            rel = [rope_release(g_ps[g]) for g in range(5)]
                            rope_math(*rel[4], kTs[tch][0:HH, :],
                                      kTs[tch][HH:H, :], cs, sn)
                            for i in range(NHC):
                                rope_math(*rel[i], qTs[tch][0:HH, i, :],
                                          qTs[tch][HH:H, i, :], cs, sn)
                    # ---------------- phase 2+3: attention + o-projection --------
                    # Attention per (q-chunk, head), all matmuls with 512-wide
                    # moving operands (fp32r full speed):
                    #   scores:  sT[s-tile, t512] = kT_tile.T @ qT_chunk
                    #   exp (+causal 0/1 mask on the diagonal band) -> pT2
                    #   AV:      avT[H, t512]    += v_tile.T(lhsT=v natural) @ pT2
                    #   denom:   l[1, t512]      += ones.T @ pT2
                    #   normalize: outT = avT * (1/l) broadcast over partitions
                    #              (1/l broadcast via a DRAM roundtrip DMA)
                    with ExitStack() as ph2:
                        # ppool/p2pool first: they should claim addresses in
                        # the early-released weight region, not the
                        # late-released rope staging region
                        ppool = ph2.enter_context(tc.tile_pool(name="ppool", bufs=2))
                        p2pool = ph2.enter_context(tc.tile_pool(name="p2pool", bufs=3))
                        wpool2 = ph2.enter_context(tc.tile_pool(name="wpool2", bufs=1))
                        otpool = ph2.enter_context(tc.tile_pool(name="otpool", bufs=2))
                        small = ph2.enter_context(tc.tile_pool(name="small", bufs=2))
                        opool = ph2.enter_context(tc.tile_pool(name="opool", bufs=2))
                        ps_s = ph2.enter_context(
                            tc.tile_pool(name="ps_s", bufs=2, space="PSUM"))
                        ps_av = ph2.enter_context(
                            tc.tile_pool(name="ps_av", bufs=2, space="PSUM"))
                        ps_l = ph2.enter_context(
                            tc.tile_pool(name="ps_l", bufs=1, space="PSUM"))
                        ps_o = ph2.enter_context(
                            tc.tile_pool(name="ps_o", bufs=2, space="PSUM"))
                        ps_bc = ph2.enter_context(
                            tc.tile_pool(name="ps_bc", bufs=1, space="PSUM"))

                        wo_sb = wpool2.tile([H, NHC, D], f32r)
                        wo_src = wo.rearrange("h p d -> p h d").bitcast(f32r)
                        for dc8 in range(8):
                            sl = slice(dc8 * TCH, (dc8 + 1) * TCH)
                            nc.sync.dma_start(out=wo_sb[:, :, sl],
                                              in_=wo_src[:, :, sl])


                        NSUB = TCH // H  # 4 t-subtiles per q-chunk

                        def emit_oproj(q0_prev, outT_prev):
                            for u in range(NSUB):
                                trow = tb + q0_prev + u * H
                                for dc in range(D // TCH):
                                    ops = ps_o.tile([H, TCH], f32, tag="o")
                                    for h in range(NHC):
                                        nc.tensor.matmul(
                                            ops,
                                            outT_prev[:, h, u * H:(u + 1) * H],
                                            wo_sb[:, h,
                                                  dc * TCH:(dc + 1) * TCH],
                                            start=(h == 0),
                                            stop=(h == NHC - 1),
                                            skip_group_check=True)
                                    o_sb = opool.tile([H, TCH], f32, tag="osb")
                                    nc.scalar.activation(
                                        o_sb, ops,
                                        mybir.ActivationFunctionType.Copy)
                                    nc.sync.dma_start(
                                        out=o_part[trow:trow + H,
                                                   dc * TCH:(dc + 1) * TCH],
                                        in_=o_sb)

                        # o-projection of q-chunk N is emitted after the first
                        # head of q-chunk N+1, hiding the normalize tail.
                        pending = None
                        for qc in range(NTCH):
                            q0 = qc * TCH
                            n_st = (qc + 1) * NSUB
                            outT_sb = otpool.tile([H, NHC, TCH], f32r, tag="outT")
                            for h in range(NHC):
                                rhs_q = qTs[qc][:, h, :]
                                av_ps = ps_av.tile([H, TCH], f32, tag="av")
                                l_ps = ps_l.tile([1, TCH], f32, tag="l")

                                def scores_block(st):
                                    sps = ps_s.tile([H, TCH], f32, tag="s")
                                    kt = kTs[st // NSUB][
                                        :, (st % NSUB) * H:(st % NSUB + 1) * H]
                                    nc.tensor.matmul(sps, kt, rhs_q,
                                                     start=True, stop=True)
                                    pT = ppool.tile([H, TCH], f32, tag="p")
                                    nc.scalar.activation(pT, sps, Exp, scale=C_SM)
                                    pT2 = p2pool.tile([H, TCH], f32r, tag="p2")
                                    j = st - qc * NSUB
                                    if j >= 0:
                                        nc.vector.tensor_mul(pT2, pT, masks[j])
                                    else:
                                        nc.vector.tensor_copy(pT2, pT)
                                    return pT2

                                def av_block(st, pT2):
                                    nc.tensor.matmul(
                                        av_ps, vs[st // NSUB][:, st % NSUB, :],
                                        pT2,
                                        start=(st == 0), stop=(st == n_st - 1),
                                        skip_group_check=True)
                                    nc.tensor.matmul(
                                        l_ps, ones_col, pT2,
                                        start=(st == 0), stop=(st == n_st - 1),
                                        skip_group_check=True)

                                prev = scores_block(0)
                                for st in range(1, n_st):
                                    cur = scores_block(st)
                                    av_block(st - 1, prev)
                                    prev = cur
                                av_block(n_st - 1, prev)

                                # normalize by 1/l: broadcast l across the 128
                                # partitions with a K=1 ones matmul, then a
                                # full-width reciprocal (a [1,512] reciprocal
                                # runs on a single DVE lane, ~6x slower).
                                l_row = small.tile([1, TCH], f32r, tag="lrow")
                                nc.vector.tensor_copy(l_row, l_ps)
                                l_bc = ps_bc.tile([H, TCH], f32, tag="bc")
                                nc.tensor.matmul(l_bc, ones_row, l_row,
                                                 start=True, stop=True)
                                rl_bc = small.tile([H, TCH], f32, tag="rlbc")
                                nc.vector.reciprocal(rl_bc, l_bc)
                                nc.vector.tensor_mul(
                                    outT_sb[:, h, :], av_ps, rl_bc)
                                if h == 0 and pending is not None:
                                    emit_oproj(*pending)
                                    pending = None
                            pending = (q0, outT_sb)
                        emit_oproj(*pending)

    nc.compile()
    return nc


_NC_CACHE = None


def kernel(x, wq, wk, wv, wo, positions):
    global _NC_CACHE
    from concourse.bass_utils import run_bass_kernel_spmd

    x = np.asarray(x, dtype=np.float32)
    wq = np.asarray(wq, dtype=np.float32)
    wk = np.asarray(wk, dtype=np.float32)
    wv = np.asarray(wv, dtype=np.float32)
    wo = np.asarray(wo, dtype=np.float32)
    positions = np.asarray(positions)

    xT = np.ascontiguousarray(x.reshape(TOK, D).T)
    # rope tables, transposed: [H/2, B*T]
    fraction = 2.0 * np.arange(HH, dtype=np.float32) / H
    timescale = (THETA ** fraction).astype(np.float32)
    pos = positions.reshape(TOK).astype(np.float32)
    sinusoid = pos[None, :] / timescale[:, None]
    cosT = np.cos(sinusoid).astype(np.float32)
    sinT = np.sin(sinusoid).astype(np.float32)
    # duplicate across both partition halves (see kernel comment)
    cosT = np.ascontiguousarray(np.concatenate([cosT, cosT], axis=0))
    sinT = np.ascontiguousarray(np.concatenate([sinT, sinT], axis=0))

    if _NC_CACHE is None:
        _NC_CACHE = _build_bass()
    nc = _NC_CACHE

    in_maps = []
    for c in range(NCORES):
        in_maps.append({
            "xT": xT,
            "wq": np.ascontiguousarray(wq[c * NHC:(c + 1) * NHC]),
            "wk": np.ascontiguousarray(wk[c]),
            "wv": np.ascontiguousarray(wv[c]),
            "wo": np.ascontiguousarray(wo[c * NHC:(c + 1) * NHC]),
            "cosT": cosT,
            "sinT": sinT,
        })

    trace = os.environ.get("BASS_KERNEL_TRACE", "0") == "1"
    res = run_bass_kernel_spmd(nc, in_maps, list(range(NCORES)), trace=trace)
    global LAST_RESULTS
    LAST_RESULTS = res
    out = np.zeros((TOK, D), dtype=np.float32)
    for c in range(NCORES):
        out += res.results[c]["o_part"]
    return out.reshape(B, T, D)


LAST_RESULTS = None



# revision 4
# speedup vs baseline: 1.3423x; 1.3423x over previous
"""GQA causal-attention prefill kernel for Trainium2, tensor-parallel over 8 NeuronCores.

Reference semantics: q/k/v projections + RoPE + causal GQA attention +
output projection, B=2, T=2048, D=4096, 32 q heads, 8 kv heads, head_dim
128.  Core c owns q heads [4c, 4c+4), kv head c and the matching wo
slice; each core computes a full-shape partial output o_part and the
host sums the 8 partials (the tensor-parallel all-reduce).

Everything on the PE runs in bf16 (fp32 PSUM accumulation); measured
end-to-end max-rel error vs the fp32 reference is ~4e-3, well inside the
2e-2 gate, and bf16 halves DMA traffic, halves SBUF footprint (so all
weights + both batches' activations stay resident) and unlocks the
2-4x DVE 16-bit modes for the softmax bookkeeping.

Structure (emission order = engine program order):
  P1(b0), P1(b1):  projections + rope, TWO passes per batch over x
      (pass A: q0,q1,k; pass B: q2,q3,v).  3 accumulation groups x
      bufs=2 PSUM banks -> evictions of chunk c overlap the full 20us
      K-sweep of chunk c+1, so the PE never waits on a bank.  x is read
      twice (bf16 makes the 2x stream fit in HBM bandwidth); weights
      are loaded once up front, in k-group tiles so the first matmul
      only waits for ~1.5MB.
  P2(b0), P2(b1):  attention + o-projection per 512-token q-chunk.
      Scores transposed (sT = kT.T @ qT) so AV contracts s on the
      partition dim.  Softmax denominator comes from DVE adds of the
      exp tiles (off the PE) + ONE all-ones [128,128] matmul per
      (chunk, head) that sums over partitions AND broadcasts in one
      shot; 1/l via the fast custom-DVE reciprocal.  q-chunks are
      processed in pairs {3,0},{2,1} with heads interleaved so each
      stream's finalize chain hides behind a long stream's matmuls,
      and the o-projection of finished chunks is emitted between
      streams to keep the PE queue deep.
"""

import os
import sys

sys.path.insert(0, "/opt/trn_rl_repo")

import numpy as np

B = 2
T = 2048
TOK = B * T
D = 4096
NQ = 32
NKV = 8
H = 128
HH = H // 2
THETA = 10000.0
NCORES = 8
NHC = NQ // NCORES          # q heads per core (4)
KPC = D // H                # contraction chunks of 128 over D (32)
KG = 4                      # k-groups per weight tensor (8 chunks each)
TCH = 512                   # token chunk
NTCH = T // TCH             # 4 token chunks per batch
NSUB = TCH // H             # 4 s-subtiles per chunk
C_SM = 1.0 / np.sqrt(H)     # softmax scale


def _build_bass():
    import concourse.bacc as bacc
    import concourse.mybir as mybir
    import concourse.tile as tile
    from concourse.masks import make_identity
    from contextlib import ExitStack

    f32 = mybir.dt.float32
    bf16 = mybir.dt.bfloat16
    Exp = mybir.ActivationFunctionType.Exp
    Copy = mybir.ActivationFunctionType.Copy

    nc = bacc.Bacc("TRN2", target_bir_lowering=False, debug=False,
                   num_devices=NCORES)

    xT = nc.declare_dram_parameter("xT", [D, TOK], bf16, isOutput=False)
    # host pre-shuffled so every DMA row is >=2KB contiguous:
    # wqs[p, h, c, m] = wq[h, c*128+p, m]
    wqs = nc.declare_dram_parameter("wqs", [H, NHC, KPC, H], bf16,
                                    isOutput=False)
    wks = nc.declare_dram_parameter("wks", [H, KPC, H], bf16, isOutput=False)
    wvs = nc.declare_dram_parameter("wvs", [H, KPC, H], bf16, isOutput=False)
    # wos[p, h, d] = wo[h, p, d]
    wos = nc.declare_dram_parameter("wos", [H, NHC, D], bf16, isOutput=False)
    # rope tables duplicated across partition halves; sinT's TOP half is
    # NEGATED on the host so rope is out = direct*cosT + swap*sinT for all
    # 128 partitions in one mul+mul+add.
    cosT = nc.declare_dram_parameter("cosT", [H, TOK], bf16, isOutput=False)
    sinT = nc.declare_dram_parameter("sinT", [H, TOK], bf16, isOutput=False)
    o_part = nc.declare_dram_parameter("o_part", [TOK, D], f32, isOutput=True)

    with tile.TileContext(nc) as tc:
        with ExitStack() as top:
            consts = top.enter_context(tc.tile_pool(name="consts", bufs=1))
            identity = consts.tile([H, H], bf16)
            make_identity(nc, identity)
            ones128 = consts.tile([H, H], bf16, tag="ones128")
            nc.vector.memset(ones128, 1.0)
            # 0/1 causal wedge masks: mask[j][s, t] = 1 iff (t - s - 128*j) >= 0
            masks = []
            for j in range(NSUB):
                m = consts.tile([H, TCH], bf16, tag=f"mask{j}",
                                name=f"mask{j}")
                nc.vector.memset(m, 1.0)
                nc.gpsimd.affine_select(
                    out=m, in_=m,
                    compare_op=mybir.AluOpType.is_ge,
                    fill=0.0,
                    base=-H * j,
                    pattern=[[1, TCH]],
                    channel_multiplier=-1,
                )
                masks.append(m)

            # ---- weights: loaded once, staged so x streaming stays ahead ----
            wpool = top.enter_context(tc.tile_pool(name="wpool", bufs=1))
            wq_t = [[wpool.tile([H, 8, H], bf16, tag=f"wq{h}_{g}",
                                name=f"wq{h}_{g}") for g in range(KG)]
                    for h in range(NHC)]
            wk_t = [wpool.tile([H, 8, H], bf16, tag=f"wk{g}", name=f"wk{g}")
                    for g in range(KG)]
            wv_t = [wpool.tile([H, 8, H], bf16, tag=f"wv{g}", name=f"wv{g}")
                    for g in range(KG)]
            wo_t = [wpool.tile([H, NHC, 1024], bf16, tag=f"wo{dq}",
                               name=f"wo{dq}") for dq in range(4)]
            cos_t = [wpool.tile([H, T], bf16, tag=f"cos{b}", name=f"cos{b}")
                     for b in range(B)]
            sin_t = [wpool.tile([H, T], bf16, tag=f"sin{b}", name=f"sin{b}")
                     for b in range(B)]

            # immediately needed: pass-A k-group 0 + batch-0 rope tables
            nc.sync.dma_start(out=wq_t[0][0], in_=wqs[:, 0, 0:8, :])
            nc.sync.dma_start(out=wq_t[1][0], in_=wqs[:, 1, 0:8, :])
            nc.sync.dma_start(out=wk_t[0], in_=wks[:, 0:8, :])
            nc.sync.dma_start(out=cos_t[0], in_=cosT[:, 0:T])
            nc.sync.dma_start(out=sin_t[0], in_=sinT[:, 0:T])
            pend = []
            for g in range(1, KG):
                pend.append((wq_t[0][g], wqs[:, 0, g * 8:(g + 1) * 8, :]))
                pend.append((wq_t[1][g], wqs[:, 1, g * 8:(g + 1) * 8, :]))
                pend.append((wk_t[g], wks[:, g * 8:(g + 1) * 8, :]))
            for g in range(KG):
                pend.append((wq_t[2][g], wqs[:, 2, g * 8:(g + 1) * 8, :]))
                pend.append((wq_t[3][g], wqs[:, 3, g * 8:(g + 1) * 8, :]))
                pend.append((wv_t[g], wvs[:, g * 8:(g + 1) * 8, :]))
            pend.append((cos_t[1], cosT[:, T:TOK]))
            pend.append((sin_t[1], sinT[:, T:TOK]))
            for dq in range(4):
                pend.append((wo_t[dq], wos[:, :, dq * 1024:(dq + 1) * 1024]))

            def drain_pend(n):
                for _ in range(n):
                    if pend:
                        dst, src = pend.pop(0)
                        nc.sync.dma_start(out=dst, in_=src)

            # ---- activations, both batches resident (bf16) ----
            apool = top.enter_context(tc.tile_pool(name="apool", bufs=1))
            qTs = [[apool.tile([H, NHC, TCH], bf16, tag=f"qT{b}_{i}",
                               name=f"qT{b}_{i}") for i in range(NTCH)]
                   for b in range(B)]
            kTs = [[apool.tile([H, TCH], bf16, tag=f"kT{b}_{i}",
                               name=f"kT{b}_{i}") for i in range(NTCH)]
                   for b in range(B)]
            vs = [[apool.tile([H, NSUB, H], bf16, tag=f"v{b}_{i}",
                              name=f"v{b}_{i}") for i in range(NTCH)]
                  for b in range(B)]

            # ================= phase 1: projections + rope =================
            with ExitStack() as ph1:
                xpool = ph1.enter_context(tc.tile_pool(name="xpool", bufs=8))
                rtmp = ph1.enter_context(tc.tile_pool(name="rtmp", bufs=2))
                pj = ph1.enter_context(
                    tc.tile_pool(name="pj", bufs=2, space="PSUM"))
                pt = ph1.enter_context(
                    tc.tile_pool(name="pt", bufs=2, space="PSUM"))

                def rope_from_psum(psum, dst_ap, cs, sn):
                    # swap staging: halves exchanged so the mul against the
                    # (half-duplicated) rope table is one full-width op.
                    swap = rtmp.tile([H, TCH], f32, tag="swap", bufs=3,
                                     name="swap")
                    nc.vector.tensor_copy(swap[0:HH, :], psum[HH:H, :])
                    nc.vector.tensor_copy(swap[HH:H, :], psum[0:HH, :])
                    m1 = rtmp.tile([H, TCH], f32, tag="m1", name="m1")
                    m2 = rtmp.tile([H, TCH], f32, tag="m2", name="m2")
                    nc.vector.tensor_mul(m1, psum, cs)
                    nc.vector.tensor_mul(m2, swap, sn)
                    nc.vector.tensor_add(dst_ap, m1, m2)

                # staged weight-DMA drain counts per (pass, chunk) of batch
                # 0: pass-A rest up front, pass-B spread over pass A, wo +
                # batch-1 rope tables spread over pass B.  Emission precedes
                # every consumer (Tile deps follow emission order).
                drains = {0: [12, 3, 3, 3], 1: [2, 2, 2, 0]}
                for b in range(B):
                    tb = b * T
                    for pas in range(2):
                        for tch in range(NTCH):
                            if b == 0:
                                drain_pend(drains[pas][tch])
                            t0 = tch * TCH
                            g_ps = [pj.tile([H, TCH], f32, tag=f"g{i}",
                                            name=f"g{i}") for i in range(3)]
                            for k in range(KPC):
                                x_t = xpool.tile([H, TCH], bf16, tag="x",
                                                 name="x_t")
                                nc.sync.dma_start(
                                    out=x_t,
                                    in_=xT[k * H:(k + 1) * H,
                                           tb + t0:tb + t0 + TCH])
                                kg, ko = k // 8, k % 8
                                if pas == 0:
                                    lhs = [wq_t[0][kg][:, ko, :],
                                           wq_t[1][kg][:, ko, :],
                                           wk_t[kg][:, ko, :]]
                                else:
                                    lhs = [wq_t[2][kg][:, ko, :],
                                           wq_t[3][kg][:, ko, :],
                                           wv_t[kg][:, ko, :]]
                                for gi in range(3):
                                    nc.tensor.matmul(
                                        g_ps[gi], lhs[gi], x_t,
                                        start=(k == 0), stop=(k == KPC - 1),
                                        skip_group_check=True)
                            cs = cos_t[b][:, t0:t0 + TCH]
                            sn = sin_t[b][:, t0:t0 + TCH]
                            if pas == 0:
                                rope_from_psum(g_ps[2], kTs[b][tch], cs, sn)
                                rope_from_psum(g_ps[0], qTs[b][tch][:, 0, :],
                                               cs, sn)
                                rope_from_psum(g_ps[1], qTs[b][tch][:, 1, :],
                                               cs, sn)
                            else:
                                vstage = rtmp.tile([H, TCH], bf16,
                                                   tag="vstage", name="vstage")
                                nc.vector.tensor_copy(vstage, g_ps[2])
                                for j in range(NSUB):
                                    tp = pt.tile([H, H], bf16, tag="vtp",
                                                 name="vtp")
                                    nc.tensor.transpose(
                                        tp, vstage[:, j * H:(j + 1) * H],
                                        identity)
                                    nc.vector.tensor_copy(
                                        vs[b][tch][:, j, :], tp)
                                rope_from_psum(g_ps[0], qTs[b][tch][:, 2, :],
                                               cs, sn)
                                rope_from_psum(g_ps[1], qTs[b][tch][:, 3, :],
                                               cs, sn)

            # ============= phase 2: attention + o-projection =============
            with ExitStack() as ph2:
                ppool = ph2.enter_context(tc.tile_pool(name="ppool", bufs=2))
                p2pool = ph2.enter_context(tc.tile_pool(name="p2pool", bufs=4))
                lpool = ph2.enter_context(tc.tile_pool(name="lpool", bufs=2))
                rpool = ph2.enter_context(tc.tile_pool(name="rpool", bufs=2))
                otpool = ph2.enter_context(tc.tile_pool(name="otpool", bufs=1))
                opool = ph2.enter_context(tc.tile_pool(name="opool", bufs=2))
                ps_s = ph2.enter_context(
                    tc.tile_pool(name="ps_s", bufs=2, space="PSUM"))
                ps_av = ph2.enter_context(
                    tc.tile_pool(name="ps_av", bufs=2, space="PSUM"))
                ps_lbc = ph2.enter_context(
                    tc.tile_pool(name="ps_lbc", bufs=2, space="PSUM"))
                ps_o = ph2.enter_context(
                    tc.tile_pool(name="ps_o", bufs=2, space="PSUM"))

                for b in range(B):
                    tb = b * T
                    outTs = {qc: otpool.tile([H, NHC, TCH], bf16,
                                             tag=f"outT{qc}",
                                             name=f"outT{qc}")
                             for qc in range(NTCH)}

                    def attn_stream(qc, h):
                        n_st = (qc + 1) * NSUB
                        rhs_q = qTs[b][qc][:, h, :]
                        av_ps = ps_av.tile([H, TCH], f32, tag="av",
                                           name="av_ps")
                        lsum = lpool.tile([H, TCH], bf16, tag="lsum",
                                          name="lsum")

                        def scores_block(st):
                            sps = ps_s.tile([H, TCH], f32, tag="s",
                                            name="sps")
                            kt = kTs[b][st // NSUB][
                                :, (st % NSUB) * H:(st % NSUB + 1) * H]
                            nc.tensor.matmul(sps, kt, rhs_q,
                                             start=True, stop=True)
                            j = st - qc * NSUB
                            pT2 = p2pool.tile([H, TCH], bf16, tag="p2",
                                              name="pT2")
                            if j >= 0:
                                pT = ppool.tile([H, TCH], bf16, tag="p",
                                                name="pT")
                                nc.scalar.activation(pT, sps, Exp,
                                                     scale=C_SM)
                                nc.vector.tensor_mul(pT2, pT, masks[j])
                            else:
                                nc.scalar.activation(pT2, sps, Exp,
                                                     scale=C_SM)
                            # softmax denominator accumulates on DVE, off
                            # the PE's critical path
                            if st == 0:
                                nc.vector.tensor_copy(lsum, pT2)
                            else:
                                nc.vector.tensor_add(lsum, lsum, pT2)
                            return pT2

                        def av_block(st, pT2):
                            nc.tensor.matmul(
                                av_ps, vs[b][st // NSUB][:, st % NSUB, :],
                                pT2,
                                start=(st == 0), stop=(st == n_st - 1),
                                skip_group_check=True)

                        prev = scores_block(0)
                        for st in range(1, n_st):
                            cur = scores_block(st)
                            av_block(st - 1, prev)
                            prev = cur
                        av_block(n_st - 1, prev)
                        # partition-sum + broadcast of the denominator in one
                        # all-ones matmul, then fast reciprocal + normalize
                        lbc = ps_lbc.tile([H, TCH], f32, tag="lbc",
                                          name="lbc")
                        nc.tensor.matmul(lbc, ones128, lsum,
                                         start=True, stop=True)
                        rl = rpool.tile([H, TCH], f32, tag="rl", name="rl")
                        nc.vector.reciprocal_approx_fast(out=rl, in_=lbc)
                        nc.vector.tensor_mul(outTs[qc][:, h, :], av_ps, rl)

                    def emit_oproj(qc):
                        outT = outTs[qc]
                        for u in range(NSUB):
                            trow = tb + qc * TCH + u * H
                            for dc in range(D // TCH):
                                ops = ps_o.tile([H, TCH], f32, tag="o",
                                                name="ops")
                                for h in range(NHC):
                                    nc.tensor.matmul(
                                        ops, outT[:, h, u * H:(u + 1) * H],
                                        wo_t[dc // 2][:, h,
                                                      (dc % 2) * TCH:
                                                      (dc % 2 + 1) * TCH],
                                        start=(h == 0), stop=(h == NHC - 1),
                                        skip_group_check=True)
                                o_sb = opool.tile([H, TCH], f32, tag="osb",
                                                  name="o_sb")
                                nc.scalar.activation(o_sb, ops, Copy)
                                nc.sync.dma_start(
                                    out=o_part[trow:trow + H,
                                               dc * TCH:(dc + 1) * TCH],
                                    in_=o_sb)

                    streams = []
                    for qa, qb in ((3, 0), (2, 1)):
                        for h in range(NHC):
                            streams.append((qa, h))
                            streams.append((qb, h))
                    for i, (qc, h) in enumerate(streams):
                        attn_stream(qc, h)
                        if i == 8:
                            emit_oproj(3)
                        elif i == 9:
                            emit_oproj(0)
                    emit_oproj(2)
                    emit_oproj(1)

    nc.compile()
    return nc


_NC_CACHE = None


def _prep_inputs(x, wq, wk, wv, wo, positions):
    import ml_dtypes
    bf = ml_dtypes.bfloat16

    x = np.asarray(x, dtype=np.float32)
    wq = np.asarray(wq, dtype=np.float32)
    wk = np.asarray(wk, dtype=np.float32)
    wv = np.asarray(wv, dtype=np.float32)
    wo = np.asarray(wo, dtype=np.float32)
    positions = np.asarray(positions)

    xT = np.ascontiguousarray(x.reshape(TOK, D).T.astype(bf))
    # rope tables [H, TOK], duplicated across halves, sin top half negated
    fraction = 2.0 * np.arange(HH, dtype=np.float32) / H
    timescale = (THETA ** fraction).astype(np.float32)
    pos = positions.reshape(TOK).astype(np.float32)
    sinu = pos[None, :] / timescale[:, None]
    cos = np.cos(sinu).astype(np.float32)
    sin = np.sin(sinu).astype(np.float32)
    cosT = np.ascontiguousarray(np.concatenate([cos, cos], 0).astype(bf))
    sinT = np.ascontiguousarray(np.concatenate([-sin, sin], 0).astype(bf))

    in_maps = []
    for c in range(NCORES):
        wq_c = wq[c * NHC:(c + 1) * NHC]            # [4, D, H]
        wqs = np.ascontiguousarray(
            wq_c.reshape(NHC, KPC, H, H).transpose(2, 0, 1, 3).astype(bf))
        wks = np.ascontiguousarray(
            wk[c].reshape(KPC, H, H).transpose(1, 0, 2).astype(bf))
        wvs = np.ascontiguousarray(
            wv[c].reshape(KPC, H, H).transpose(1, 0, 2).astype(bf))
        wos = np.ascontiguousarray(
            wo[c * NHC:(c + 1) * NHC].transpose(1, 0, 2).astype(bf))
        in_maps.append({
            "xT": xT,
            "wqs": wqs,
            "wks": wks,
            "wvs": wvs,
            "wos": wos,
            "cosT": cosT,
            "sinT": sinT,
        })
    return in_maps


def kernel(x, wq, wk, wv, wo, positions):
    global _NC_CACHE
    from concourse.bass_utils import run_bass_kernel_spmd

    in_maps = _prep_inputs(x, wq, wk, wv, wo, positions)

    if _NC_CACHE is None:
        _NC_CACHE = _build_bass()
    nc = _NC_CACHE

    trace = os.environ.get("BASS_KERNEL_TRACE", "0") == "1"
    res = run_bass_kernel_spmd(nc, in_maps, list(range(NCORES)), trace=trace)
    global LAST_RESULTS
    LAST_RESULTS = res
    out = np.zeros((TOK, D), dtype=np.float32)
    for c in range(NCORES):
        out += np.asarray(res.results[c]["o_part"], dtype=np.float32)
    return out.reshape(B, T, D)


LAST_RESULTS = None


# revision 9
# speedup vs baseline: 1.5086x; 1.1239x over previous
"""GQA causal-attention prefill kernel for Trainium2, tensor-parallel over 8 NeuronCores.

Reference semantics: q/k/v projections + RoPE + causal GQA attention +
output projection, B=2, T=2048, D=4096, 32 q heads, 8 kv heads, head_dim
128.  Core c owns q heads [4c, 4c+4), kv head c and the matching wo
slice; each core computes a full-shape partial output o_part and the
host sums the 8 partials (the tensor-parallel all-reduce).

Everything on the PE runs in bf16 (fp32 PSUM accumulation); measured
end-to-end max-rel error vs the fp32 reference is ~4e-3, well inside the
2e-2 gate, and bf16 halves DMA traffic, halves SBUF footprint (so all
weights + both batches' activations stay resident) and unlocks the
2-4x DVE 16-bit modes for the softmax bookkeeping.

Structure (emission order = engine program order):
  P1(b0), P1(b1):  projections + rope, TWO passes per batch over x
      (pass A: q0,q1,k; pass B: q2,q3,v).  3 accumulation groups x
      bufs=2 PSUM banks -> evictions of chunk c overlap the full 20us
      K-sweep of chunk c+1, so the PE never waits on a bank.  x is read
      twice (bf16 makes the 2x stream fit in HBM bandwidth); weights
      are loaded once up front, in k-group tiles so the first matmul
      only waits for ~1.5MB.
  P2(b0), P2(b1):  attention + o-projection per 512-token q-chunk.
      Scores transposed (sT = kT.T @ qT) so AV contracts s on the
      partition dim.  Softmax denominator comes from DVE adds of the
      exp tiles (off the PE) + ONE all-ones [128,128] matmul per
      (chunk, head) that sums over partitions AND broadcasts in one
      shot; 1/l via the fast custom-DVE reciprocal.  q-chunks are
      processed in pairs {3,0},{2,1} with heads interleaved so each
      stream's finalize chain hides behind a long stream's matmuls,
      and the o-projection of finished chunks is emitted between
      streams to keep the PE queue deep.
"""

import os
import sys

sys.path.insert(0, "/opt/trn_rl_repo")

import numpy as np

B = 2
T = 2048
TOK = B * T
D = 4096
NQ = 32
NKV = 8
H = 128
HH = H // 2
THETA = 10000.0
NCORES = 8
NHC = NQ // NCORES          # q heads per core (4)
KPC = D // H                # contraction chunks of 128 over D (32)
KG = 4                      # k-groups per weight tensor (8 chunks each)
TCH = 512                   # token chunk
NTCH = T // TCH             # 4 token chunks per batch
NSUB = TCH // H             # 4 s-subtiles per chunk
C_SM = 1.0 / np.sqrt(H)     # softmax scale


def _build_bass():
    import concourse.bacc as bacc
    import concourse.mybir as mybir
    import concourse.tile as tile
    from concourse.masks import make_identity
    from contextlib import ExitStack

    f32 = mybir.dt.float32
    bf16 = mybir.dt.bfloat16
    Exp = mybir.ActivationFunctionType.Exp
    Copy = mybir.ActivationFunctionType.Copy

    nc = bacc.Bacc("TRN2", target_bir_lowering=False, debug=False,
                   num_devices=NCORES)

    xT = nc.declare_dram_parameter("xT", [D, TOK], bf16, isOutput=False)
    # host pre-shuffled so every DMA row is >=2KB contiguous:
    # wqs[p, h, c, m] = wq[h, c*128+p, m]
    wqs = nc.declare_dram_parameter("wqs", [H, NHC, KPC, H], bf16,
                                    isOutput=False)
    wks = nc.declare_dram_parameter("wks", [H, KPC, H], bf16, isOutput=False)
    wvs = nc.declare_dram_parameter("wvs", [H, KPC, H], bf16, isOutput=False)
    # wos[p, h, d] = wo[h, p, d]
    wos = nc.declare_dram_parameter("wos", [H, NHC, D], bf16, isOutput=False)
    # rope tables duplicated across partition halves; sinT's TOP half is
    # NEGATED on the host so rope is out = direct*cosT + swap*sinT for all
    # 128 partitions in one mul+mul+add.
    cosT = nc.declare_dram_parameter("cosT", [H, TOK], bf16, isOutput=False)
    sinT = nc.declare_dram_parameter("sinT", [H, TOK], bf16, isOutput=False)
    o_part = nc.declare_dram_parameter("o_part", [TOK, D], bf16, isOutput=True)
    # x viewed as [p, kchunk, t] so one DMA start can fetch 4 k-chunks
    # (each dma_start costs ~600ns of serial Sync-sequencer time; the
    # un-batched version saturated that queue)
    xTv = xT.rearrange("(c p) t -> p c t", p=H)

    with tile.TileContext(nc) as tc:
        with ExitStack() as top:
            consts = top.enter_context(tc.tile_pool(name="consts", bufs=1))
            identity = consts.tile([H, H], bf16)
            make_identity(nc, identity)
            ones128 = consts.tile([H, H], bf16, tag="ones128")
            nc.vector.memset(ones128, 1.0)
            # 0/1 causal wedge masks: mask[j][s, t] = 1 iff (t - s - 128*j) >= 0
            masks = []
            for j in range(NSUB):
                m = consts.tile([H, TCH], bf16, tag=f"mask{j}",
                                name=f"mask{j}")
                nc.vector.memset(m, 1.0)
                nc.gpsimd.affine_select(
                    out=m, in_=m,
                    compare_op=mybir.AluOpType.is_ge,
                    fill=0.0,
                    base=-H * j,
                    pattern=[[1, TCH]],
                    channel_multiplier=-1,
                )
                masks.append(m)

            # ---- weights: loaded once, staged so x streaming stays ahead ----
            wpool = top.enter_context(tc.tile_pool(name="wpool", bufs=1))
            wq_t = [[wpool.tile([H, 8, H], bf16, tag=f"wq{h}_{g}",
                                name=f"wq{h}_{g}") for g in range(KG)]
                    for h in range(NHC)]
            wk_t = [wpool.tile([H, 8, H], bf16, tag=f"wk{g}", name=f"wk{g}")
                    for g in range(KG)]
            wv_t = [wpool.tile([H, 8, H], bf16, tag=f"wv{g}", name=f"wv{g}")
                    for g in range(KG)]
            wo_t = [wpool.tile([H, NHC, 1024], bf16, tag=f"wo{dq}",
                               name=f"wo{dq}") for dq in range(4)]
            cos_t = [wpool.tile([H, T], bf16, tag=f"cos{b}", name=f"cos{b}")
                     for b in range(B)]
            sin_t = [wpool.tile([H, T], bf16, tag=f"sin{b}", name=f"sin{b}")
                     for b in range(B)]

            # immediately needed: pass-A k-group 0 + batch-0 rope tables
            nc.sync.dma_start(out=wq_t[0][0], in_=wqs[:, 0, 0:8, :])
            nc.sync.dma_start(out=wq_t[1][0], in_=wqs[:, 1, 0:8, :])
            nc.sync.dma_start(out=wk_t[0], in_=wks[:, 0:8, :])
            nc.sync.dma_start(out=cos_t[0], in_=cosT[:, 0:T])
            nc.sync.dma_start(out=sin_t[0], in_=sinT[:, 0:T])
            pend = []
            for g in range(1, KG):
                pend.append((wq_t[0][g], wqs[:, 0, g * 8:(g + 1) * 8, :]))
                pend.append((wq_t[1][g], wqs[:, 1, g * 8:(g + 1) * 8, :]))
                pend.append((wk_t[g], wks[:, g * 8:(g + 1) * 8, :]))
            for g in range(KG):
                pend.append((wq_t[2][g], wqs[:, 2, g * 8:(g + 1) * 8, :]))
                pend.append((wq_t[3][g], wqs[:, 3, g * 8:(g + 1) * 8, :]))
                pend.append((wv_t[g], wvs[:, g * 8:(g + 1) * 8, :]))
            pend.append((cos_t[1], cosT[:, T:TOK]))
            pend.append((sin_t[1], sinT[:, T:TOK]))
            for dq in range(4):
                pend.append((wo_t[dq], wos[:, :, dq * 1024:(dq + 1) * 1024]))

            def drain_pend(n):
                for _ in range(n):
                    if pend:
                        dst, src = pend.pop(0)
                        nc.sync.dma_start(out=dst, in_=src)

            # ---- activations, both batches resident (bf16) ----
            apool = top.enter_context(tc.tile_pool(name="apool", bufs=1))
            qTs = [[apool.tile([H, NHC, TCH], bf16, tag=f"qT{b}_{i}",
                               name=f"qT{b}_{i}") for i in range(NTCH)]
                   for b in range(B)]
            kTs = [[apool.tile([H, TCH], bf16, tag=f"kT{b}_{i}",
                               name=f"kT{b}_{i}") for i in range(NTCH)]
                   for b in range(B)]
            vs = [[apool.tile([H, NSUB, H], bf16, tag=f"v{b}_{i}",
                              name=f"v{b}_{i}") for i in range(NTCH)]
                  for b in range(B)]

            # ================= phase 1: projections + rope =================
            with ExitStack() as ph1:
                xpool = ph1.enter_context(tc.tile_pool(name="xpool", bufs=4))
                rtmp = ph1.enter_context(tc.tile_pool(name="rtmp", bufs=2))
                pj = ph1.enter_context(
                    tc.tile_pool(name="pj", bufs=2, space="PSUM"))
                pt = ph1.enter_context(
                    tc.tile_pool(name="pt", bufs=2, space="PSUM"))

                def rope_from_psum(psum, dst_ap, cs, sn):
                    # swap staging: halves exchanged so the mul against the
                    # (half-duplicated) rope table is one full-width op.
                    swap = rtmp.tile([H, TCH], f32, tag="swap", bufs=3,
                                     name="swap")
                    nc.vector.tensor_copy(swap[0:HH, :], psum[HH:H, :])
                    nc.vector.tensor_copy(swap[HH:H, :], psum[0:HH, :])
                    m1 = rtmp.tile([H, TCH], f32, tag="m1", name="m1")
                    m2 = rtmp.tile([H, TCH], f32, tag="m2", name="m2")
                    nc.vector.tensor_mul(m1, psum, cs)
                    nc.vector.tensor_mul(m2, swap, sn)
                    nc.vector.tensor_add(dst_ap, m1, m2)

                # staged weight-DMA drain counts per (pass, chunk) of batch
                # 0: pass-A rest up front, pass-B spread over pass A, wo +
                # batch-1 rope tables spread over pass B.  Emission precedes
                # every consumer (Tile deps follow emission order).
                drains = {0: [12, 3, 3, 3], 1: [2, 2, 2, 0]}
                for b in range(B):
                    tb = b * T
                    for pas in range(2):
                        for tch in range(NTCH):
                            if b == 0:
                                drain_pend(drains[pas][tch])
                            t0 = tch * TCH
                            g_ps = [pj.tile([H, TCH], f32, tag=f"g{i}",
                                            name=f"g{i}") for i in range(3)]
                            for kq in range(KPC // 4):
                                x_t = xpool.tile([H, 4, TCH], bf16, tag="x",
                                                 name="x_t")
                                nc.sync.dma_start(
                                    out=x_t,
                                    in_=xTv[:, kq * 4:(kq + 1) * 4,
                                            tb + t0:tb + t0 + TCH])
                                for kc in range(4):
                                    k = kq * 4 + kc
                                    kg, ko = k // 8, k % 8
                                    if pas == 0:
                                        lhs = [wq_t[0][kg][:, ko, :],
                                               wq_t[1][kg][:, ko, :],
                                               wk_t[kg][:, ko, :]]
                                    else:
                                        lhs = [wq_t[2][kg][:, ko, :],
                                               wq_t[3][kg][:, ko, :],
                                               wv_t[kg][:, ko, :]]
                                    for gi in range(3):
                                        nc.tensor.matmul(
                                            g_ps[gi], lhs[gi], x_t[:, kc, :],
                                            start=(k == 0),
                                            stop=(k == KPC - 1),
                                            skip_group_check=True)
                            cs = cos_t[b][:, t0:t0 + TCH]
                            sn = sin_t[b][:, t0:t0 + TCH]
                            if pas == 0:
                                rope_from_psum(g_ps[2], kTs[b][tch], cs, sn)
                                rope_from_psum(g_ps[0], qTs[b][tch][:, 0, :],
                                               cs, sn)
                                rope_from_psum(g_ps[1], qTs[b][tch][:, 1, :],
                                               cs, sn)
                            else:
                                vstage = rtmp.tile([H, TCH], bf16,
                                                   tag="vstage", name="vstage")
                                nc.vector.tensor_copy(vstage, g_ps[2])
                                for j in range(NSUB):
                                    tp = pt.tile([H, H], bf16, tag="vtp",
                                                 name="vtp")
                                    nc.tensor.transpose(
                                        tp, vstage[:, j * H:(j + 1) * H],
                                        identity)
                                    nc.vector.tensor_copy(
                                        vs[b][tch][:, j, :], tp)
                                rope_from_psum(g_ps[0], qTs[b][tch][:, 2, :],
                                               cs, sn)
                                rope_from_psum(g_ps[1], qTs[b][tch][:, 3, :],
                                               cs, sn)

            # ============= phase 2: attention + o-projection =============
            with ExitStack() as ph2:
                ppool = ph2.enter_context(tc.tile_pool(name="ppool", bufs=2))
                p2pool = ph2.enter_context(tc.tile_pool(name="p2pool", bufs=4))
                lpool = ph2.enter_context(tc.tile_pool(name="lpool", bufs=2))
                rpool = ph2.enter_context(tc.tile_pool(name="rpool", bufs=2))
                otpool = ph2.enter_context(tc.tile_pool(name="otpool", bufs=1))
                opool = ph2.enter_context(tc.tile_pool(name="opool", bufs=2))
                ps_s = ph2.enter_context(
                    tc.tile_pool(name="ps_s", bufs=2, space="PSUM"))
                ps_av = ph2.enter_context(
                    tc.tile_pool(name="ps_av", bufs=2, space="PSUM"))
                ps_lbc = ph2.enter_context(
                    tc.tile_pool(name="ps_lbc", bufs=2, space="PSUM"))
                ps_o = ph2.enter_context(
                    tc.tile_pool(name="ps_o", bufs=2, space="PSUM"))

                for b in range(B):
                    tb = b * T
                    outTs = {qc: otpool.tile([H, NHC, TCH], bf16,
                                             tag=f"outT{qc}",
                                             name=f"outT{qc}")
                             for qc in range(NTCH)}

                    def attn_stream(qc, h):
                        n_st = (qc + 1) * NSUB
                        rhs_q = qTs[b][qc][:, h, :]
                        av_ps = ps_av.tile([H, TCH], f32, tag="av",
                                           name="av_ps")
                        lsum = lpool.tile([H, TCH], bf16, tag="lsum",
                                          name="lsum")

                        def scores_block(st):
                            sps = ps_s.tile([H, TCH], f32, tag="s",
                                            name="sps")
                            kt = kTs[b][st // NSUB][
                                :, (st % NSUB) * H:(st % NSUB + 1) * H]
                            nc.tensor.matmul(sps, kt, rhs_q,
                                             start=True, stop=True)
                            j = st - qc * NSUB
                            pT2 = p2pool.tile([H, TCH], bf16, tag="p2",
                                              name="pT2")
                            if j >= 0:
                                pT = ppool.tile([H, TCH], bf16, tag="p",
                                                name="pT")
                                nc.scalar.activation(pT, sps, Exp,
                                                     scale=C_SM)
                                nc.vector.tensor_mul(pT2, pT, masks[j])
                            else:
                                nc.scalar.activation(pT2, sps, Exp,
                                                     scale=C_SM)
                            # softmax denominator accumulates on DVE, off
                            # the PE's critical path
                            if st == 0:
                                nc.vector.tensor_copy(lsum, pT2)
                            else:
                                nc.vector.tensor_add(lsum, lsum, pT2)
                            return pT2

                        def av_block(st, pT2):
                            nc.tensor.matmul(
                                av_ps, vs[b][st // NSUB][:, st % NSUB, :],
                                pT2,
                                start=(st == 0), stop=(st == n_st - 1),
                                skip_group_check=True)

                        prev = scores_block(0)
                        for st in range(1, n_st):
                            cur = scores_block(st)
                            av_block(st - 1, prev)
                            prev = cur
                        av_block(n_st - 1, prev)
                        # partition-sum + broadcast of the denominator in one
                        # all-ones matmul, then fast reciprocal + normalize
                        lbc = ps_lbc.tile([H, TCH], f32, tag="lbc",
                                          name="lbc")
                        nc.tensor.matmul(lbc, ones128, lsum,
                                         start=True, stop=True)
                        rl = rpool.tile([H, TCH], f32, tag="rl", name="rl")
                        nc.vector.reciprocal_approx_fast(out=rl, in_=lbc)
                        nc.vector.tensor_mul(outTs[qc][:, h, :], av_ps, rl)

                    def emit_oproj(qc):
                        outT = outTs[qc]
                        for u in range(NSUB):
                            trow = tb + qc * TCH + u * H
                            for dh in range(2):
                                # 4 PSUM evictions batched into one 4KB-row
                                # store: keeps the Sync queue off the
                                # critical path
                                o_sb = opool.tile([H, 4, TCH], bf16,
                                                  tag="osb", name="o_sb")
                                for j in range(4):
                                    dc = dh * 4 + j
                                    ops = ps_o.tile([H, TCH], f32, tag="o",
                                                    name="ops")
                                    for h in range(NHC):
                                        nc.tensor.matmul(
                                            ops,
                                            outT[:, h, u * H:(u + 1) * H],
                                            wo_t[dc // 2][:, h,
                                                          (dc % 2) * TCH:
                                                          (dc % 2 + 1) * TCH],
                                            start=(h == 0),
                                            stop=(h == NHC - 1),
                                            skip_group_check=True)
                                    nc.scalar.activation(
                                        o_sb[:, j, :], ops, Copy)
                                nc.sync.dma_start(
                                    out=o_part[trow:trow + H,
                                               dh * 2048:(dh + 1) * 2048],
                                    in_=o_sb)

                    streams = []
                    for qa, qb in ((3, 0), (2, 1)):
                        for h in range(NHC):
                            streams.append((qa, h))
                            streams.append((qb, h))
                    for i, (qc, h) in enumerate(streams):
                        attn_stream(qc, h)
                        if i == 8:
                            emit_oproj(3)
                        elif i == 9:
                            emit_oproj(0)
                    emit_oproj(2)
                    emit_oproj(1)

    nc.compile()
    return nc


_NC_CACHE = None


def _prep_inputs(x, wq, wk, wv, wo, positions):
    import ml_dtypes
    bf = ml_dtypes.bfloat16

    x = np.asarray(x, dtype=np.float32)
    wq = np.asarray(wq, dtype=np.float32)
    wk = np.asarray(wk, dtype=np.float32)
    wv = np.asarray(wv, dtype=np.float32)
    wo = np.asarray(wo, dtype=np.float32)
    positions = np.asarray(positions)

    xT = np.ascontiguousarray(x.reshape(TOK, D).T.astype(bf))
    # rope tables [H, TOK], duplicated across halves, sin top half negated
    fraction = 2.0 * np.arange(HH, dtype=np.float32) / H
    timescale = (THETA ** fraction).astype(np.float32)
    pos = positions.reshape(TOK).astype(np.float32)
    sinu = pos[None, :] / timescale[:, None]
    cos = np.cos(sinu).astype(np.float32)
    sin = np.sin(sinu).astype(np.float32)
    cosT = np.ascontiguousarray(np.concatenate([cos, cos], 0).astype(bf))
    sinT = np.ascontiguousarray(np.concatenate([-sin, sin], 0).astype(bf))

    in_maps = []
    for c in range(NCORES):
        wq_c = wq[c * NHC:(c + 1) * NHC]            # [4, D, H]
        wqs = np.ascontiguousarray(
            wq_c.reshape(NHC, KPC, H, H).transpose(2, 0, 1, 3).astype(bf))
        wks = np.ascontiguousarray(
            wk[c].reshape(KPC, H, H).transpose(1, 0, 2).astype(bf))
        wvs = np.ascontiguousarray(
            wv[c].reshape(KPC, H, H).transpose(1, 0, 2).astype(bf))
        wos = np.ascontiguousarray(
            wo[c * NHC:(c + 1) * NHC].transpose(1, 0, 2).astype(bf))
        in_maps.append({
            "xT": xT,
            "wqs": wqs,
            "wks": wks,
            "wvs": wvs,
            "wos": wos,
            "cosT": cosT,
            "sinT": sinT,
        })
    return in_maps


def kernel(x, wq, wk, wv, wo, positions):
    global _NC_CACHE
    from concourse.bass_utils import run_bass_kernel_spmd

    in_maps = _prep_inputs(x, wq, wk, wv, wo, positions)

    if _NC_CACHE is None:
        _NC_CACHE = _build_bass()
    nc = _NC_CACHE

    trace = os.environ.get("BASS_KERNEL_TRACE", "0") == "1"
    res = run_bass_kernel_spmd(nc, in_maps, list(range(NCORES)), trace=trace)
    global LAST_RESULTS
    LAST_RESULTS = res
    out = np.zeros((TOK, D), dtype=np.float32)
    for c in range(NCORES):
        out += np.asarray(res.results[c]["o_part"]).astype(np.float32)
    return out.reshape(B, T, D)


LAST_RESULTS = None


# revision 13
# speedup vs baseline: 1.5551x; 1.0308x over previous
"""GQA causal-attention prefill kernel for Trainium2, tensor-parallel over 8 NeuronCores.

Reference semantics: q/k/v projections + RoPE + causal GQA attention +
output projection, B=2, T=2048, D=4096, 32 q heads, 8 kv heads, head_dim
128.  Core c owns q heads [4c, 4c+4), kv head c and the matching wo
slice; each core computes a full-shape partial output o_part and the
host sums the 8 partials (the tensor-parallel all-reduce).

Everything on the PE runs in bf16 (fp32 PSUM accumulation); measured
end-to-end max-rel error vs the fp32 reference is ~4e-3, well inside the
2e-2 gate, and bf16 halves DMA traffic, halves SBUF footprint (so all
weights + both batches' activations stay resident) and unlocks the
2-4x DVE 16-bit modes for the softmax bookkeeping.

Structure (emission order = engine program order):
  P1(b0), P1(b1):  projections + rope, TWO passes per batch over x
      (pass A: q0,q1,k; pass B: q2,q3,v).  3 accumulation groups x
      bufs=2 PSUM banks -> evictions of chunk c overlap the full 20us
      K-sweep of chunk c+1, so the PE never waits on a bank.  x is read
      twice (bf16 makes the 2x stream fit in HBM bandwidth); weights
      are loaded once up front, in k-group tiles so the first matmul
      only waits for ~1.5MB.
  P2(b0), P2(b1):  attention + o-projection per 512-token q-chunk.
      Scores transposed (sT = kT.T @ qT) so AV contracts s on the
      partition dim.  Softmax denominator comes from DVE adds of the
      exp tiles (off the PE) + ONE all-ones [128,128] matmul per
      (chunk, head) that sums over partitions AND broadcasts in one
      shot; 1/l via the fast custom-DVE reciprocal.  q-chunks are
      processed in pairs {3,0},{2,1} with heads interleaved so each
      stream's finalize chain hides behind a long stream's matmuls,
      and the o-projection of finished chunks is emitted between
      streams to keep the PE queue deep.
"""

import os
import sys

sys.path.insert(0, "/opt/trn_rl_repo")

import numpy as np

B = 2
T = 2048
TOK = B * T
D = 4096
NQ = 32
NKV = 8
H = 128
HH = H // 2
THETA = 10000.0
NCORES = 8
NHC = NQ // NCORES          # q heads per core (4)
KPC = D // H                # contraction chunks of 128 over D (32)
KG = 4                      # k-groups per weight tensor (8 chunks each)
TCH = 512                   # token chunk
NTCH = T // TCH             # 4 token chunks per batch
NSUB = TCH // H             # 4 s-subtiles per chunk
C_SM = 1.0 / np.sqrt(H)     # softmax scale


def _build_bass():
    import concourse.bacc as bacc
    import concourse.mybir as mybir
    import concourse.tile as tile
    from concourse.masks import make_identity
    from contextlib import ExitStack

    f32 = mybir.dt.float32
    bf16 = mybir.dt.bfloat16
    Exp = mybir.ActivationFunctionType.Exp
    Copy = mybir.ActivationFunctionType.Copy

    nc = bacc.Bacc("TRN2", target_bir_lowering=False, debug=False,
                   num_devices=NCORES)

    xT = nc.declare_dram_parameter("xT", [D, TOK], bf16, isOutput=False)
    # host pre-shuffled so every DMA row is >=2KB contiguous:
    # wqs[p, h, c, m] = wq[h, c*128+p, m]
    wqs = nc.declare_dram_parameter("wqs", [H, NHC, KPC, H], bf16,
                                    isOutput=False)
    wks = nc.declare_dram_parameter("wks", [H, KPC, H], bf16, isOutput=False)
    wvs = nc.declare_dram_parameter("wvs", [H, KPC, H], bf16, isOutput=False)
    # wos[p, h, d] = wo[h, p, d]
    wos = nc.declare_dram_parameter("wos", [H, NHC, D], bf16, isOutput=False)
    # rope tables duplicated across partition halves; sinT's TOP half is
    # NEGATED on the host so rope is out = direct*cosT + swap*sinT for all
    # 128 partitions in one mul+mul+add.
    cosT = nc.declare_dram_parameter("cosT", [H, TOK], bf16, isOutput=False)
    sinT = nc.declare_dram_parameter("sinT", [H, TOK], bf16, isOutput=False)
    o_part = nc.declare_dram_parameter("o_part", [TOK, D], bf16, isOutput=True)
    # x viewed as [p, kchunk, t] so one DMA start can fetch 4 k-chunks
    # (each dma_start costs ~600ns of serial Sync-sequencer time; the
    # un-batched version saturated that queue)
    xTv = xT.rearrange("(c p) t -> p c t", p=H)

    with tile.TileContext(nc) as tc:
        with ExitStack() as top:
            consts = top.enter_context(tc.tile_pool(name="consts", bufs=1))
            identity = consts.tile([H, H], bf16)
            make_identity(nc, identity)
            ones128 = consts.tile([H, H], bf16, tag="ones128")
            nc.vector.memset(ones128, 1.0)
            # 0/1 causal wedge masks: mask[j][s, t] = 1 iff (t - s - 128*j) >= 0
            masks = []
            for j in range(NSUB):
                m = consts.tile([H, TCH], bf16, tag=f"mask{j}",
                                name=f"mask{j}")
                nc.vector.memset(m, 1.0)
                nc.gpsimd.affine_select(
                    out=m, in_=m,
                    compare_op=mybir.AluOpType.is_ge,
                    fill=0.0,
                    base=-H * j,
                    pattern=[[1, TCH]],
                    channel_multiplier=-1,
                )
                masks.append(m)

            # ---- weights: loaded once, staged so x streaming stays ahead ----
            wpool = top.enter_context(tc.tile_pool(name="wpool", bufs=1))
            wq_t = [[wpool.tile([H, 8, H], bf16, tag=f"wq{h}_{g}",
                                name=f"wq{h}_{g}") for g in range(KG)]
                    for h in range(NHC)]
            wk_t = [wpool.tile([H, 8, H], bf16, tag=f"wk{g}", name=f"wk{g}")
                    for g in range(KG)]
            wv_t = [wpool.tile([H, 8, H], bf16, tag=f"wv{g}", name=f"wv{g}")
                    for g in range(KG)]
            wo_t = [wpool.tile([H, NHC, 1024], bf16, tag=f"wo{dq}",
                               name=f"wo{dq}") for dq in range(4)]
            cos_t = [wpool.tile([H, T], bf16, tag=f"cos{b}", name=f"cos{b}")
                     for b in range(B)]
            sin_t = [wpool.tile([H, T], bf16, tag=f"sin{b}", name=f"sin{b}")
                     for b in range(B)]

            # immediately needed: pass-A k-group 0; everything else is
            # drained between x loads so the first x tile isn't queued
            # behind megabytes of weights
            nc.sync.dma_start(out=wq_t[0][0], in_=wqs[:, 0, 0:8, :])
            nc.sync.dma_start(out=wq_t[1][0], in_=wqs[:, 1, 0:8, :])
            nc.sync.dma_start(out=wk_t[0], in_=wks[:, 0:8, :])
            pend = []
            for g in range(1, KG):
                pend.append((wq_t[0][g], wqs[:, 0, g * 8:(g + 1) * 8, :]))
                pend.append((wq_t[1][g], wqs[:, 1, g * 8:(g + 1) * 8, :]))
                pend.append((wk_t[g], wks[:, g * 8:(g + 1) * 8, :]))
            pend.append((cos_t[0], cosT[:, 0:T]))
            pend.append((sin_t[0], sinT[:, 0:T]))
            for g in range(KG):
                pend.append((wq_t[2][g], wqs[:, 2, g * 8:(g + 1) * 8, :]))
                pend.append((wq_t[3][g], wqs[:, 3, g * 8:(g + 1) * 8, :]))
                pend.append((wv_t[g], wvs[:, g * 8:(g + 1) * 8, :]))
            pend.append((cos_t[1], cosT[:, T:TOK]))
            pend.append((sin_t[1], sinT[:, T:TOK]))
            for dq in range(4):
                pend.append((wo_t[dq], wos[:, :, dq * 1024:(dq + 1) * 1024]))

            def drain_pend(n):
                for _ in range(n):
                    if pend:
                        dst, src = pend.pop(0)
                        nc.sync.dma_start(out=dst, in_=src)

            # ---- activations, both batches resident (bf16) ----
            apool = top.enter_context(tc.tile_pool(name="apool", bufs=1))
            qTs = [[apool.tile([H, NHC, TCH], bf16, tag=f"qT{b}_{i}",
                               name=f"qT{b}_{i}") for i in range(NTCH)]
                   for b in range(B)]
            kTs = [[apool.tile([H, TCH], bf16, tag=f"kT{b}_{i}",
                               name=f"kT{b}_{i}") for i in range(NTCH)]
                   for b in range(B)]
            vs = [[apool.tile([H, NSUB, H], bf16, tag=f"v{b}_{i}",
                              name=f"v{b}_{i}") for i in range(NTCH)]
                  for b in range(B)]

            # ================= phase 1: projections + rope =================
            with ExitStack() as ph1:
                xpool = ph1.enter_context(tc.tile_pool(name="xpool", bufs=4))
                rtmp = ph1.enter_context(tc.tile_pool(name="rtmp", bufs=2))
                pj = ph1.enter_context(
                    tc.tile_pool(name="pj", bufs=2, space="PSUM"))
                pt = ph1.enter_context(
                    tc.tile_pool(name="pt", bufs=2, space="PSUM"))

                def rope_from_psum(psum, dst_ap, cs, sn):
                    # swap staging: halves exchanged so the mul against the
                    # (half-duplicated) rope table is one full-width op.
                    swap = rtmp.tile([H, TCH], f32, tag="swap", bufs=3,
                                     name="swap")
                    nc.vector.tensor_copy(swap[0:HH, :], psum[HH:H, :])
                    nc.vector.tensor_copy(swap[HH:H, :], psum[0:HH, :])
                    m1 = rtmp.tile([H, TCH], f32, tag="m1", name="m1")
                    m2 = rtmp.tile([H, TCH], f32, tag="m2", name="m2")
                    nc.vector.tensor_mul(m1, psum, cs)
                    nc.vector.tensor_mul(m2, swap, sn)
                    nc.vector.tensor_add(dst_ap, m1, m2)

                # staged weight-DMA drain counts, interleaved between the
                # x loads of batch 0 (emission precedes every consumer —
                # Tile deps follow emission order; kg g's weights are
                # drained right before the x group that consumes them).
                drains = {0: [[0, 3, 3, 5], [4, 0, 0, 0], [4, 0, 0, 0],
                              [4, 0, 0, 0]],
                          1: [[2, 0, 0, 0], [2, 0, 0, 0], [2, 0, 0, 0],
                              [0, 0, 0, 0]]}
                for b in range(B):
                    tb = b * T
                    for pas in range(2):
                        for tch in range(NTCH):
                            t0 = tch * TCH
                            g_ps = [pj.tile([H, TCH], f32, tag=f"g{i}",
                                            name=f"g{i}") for i in range(3)]
                            for kq in range(KPC // 8):
                                if b == 0:
                                    drain_pend(drains[pas][tch][kq])
                                x_t = xpool.tile([H, 8, TCH], bf16, tag="x",
                                                 name="x_t")
                                nc.sync.dma_start(
                                    out=x_t,
                                    in_=xTv[:, kq * 8:(kq + 1) * 8,
                                            tb + t0:tb + t0 + TCH])
                                for kc in range(8):
                                    k = kq * 8 + kc
                                    if pas == 0:
                                        lhs = [wq_t[0][kq][:, kc, :],
                                               wq_t[1][kq][:, kc, :],
                                               wk_t[kq][:, kc, :]]
                                    else:
                                        lhs = [wq_t[2][kq][:, kc, :],
                                               wq_t[3][kq][:, kc, :],
                                               wv_t[kq][:, kc, :]]
                                    for gi in range(3):
                                        nc.tensor.matmul(
                                            g_ps[gi], lhs[gi], x_t[:, kc, :],
                                            start=(k == 0),
                                            stop=(k == KPC - 1),
                                            skip_group_check=True)
                            cs = cos_t[b][:, t0:t0 + TCH]
                            sn = sin_t[b][:, t0:t0 + TCH]
                            if pas == 0:
                                rope_from_psum(g_ps[2], kTs[b][tch], cs, sn)
                                rope_from_psum(g_ps[0], qTs[b][tch][:, 0, :],
                                               cs, sn)
                                rope_from_psum(g_ps[1], qTs[b][tch][:, 1, :],
                                               cs, sn)
                            else:
                                vstage = rtmp.tile([H, TCH], bf16,
                                                   tag="vstage", name="vstage")
                                nc.vector.tensor_copy(vstage, g_ps[2])
                                for j in range(NSUB):
                                    tp = pt.tile([H, H], bf16, tag="vtp",
                                                 name="vtp")
                                    nc.tensor.transpose(
                                        tp, vstage[:, j * H:(j + 1) * H],
                                        identity)
                                    nc.vector.tensor_copy(
                                        vs[b][tch][:, j, :], tp)
                                rope_from_psum(g_ps[0], qTs[b][tch][:, 2, :],
                                               cs, sn)
                                rope_from_psum(g_ps[1], qTs[b][tch][:, 3, :],
                                               cs, sn)

            # ============= phase 2: attention + o-projection =============
            with ExitStack() as ph2:
                ppool = ph2.enter_context(tc.tile_pool(name="ppool", bufs=2))
                p2pool = ph2.enter_context(tc.tile_pool(name="p2pool", bufs=4))
                lpool = ph2.enter_context(tc.tile_pool(name="lpool", bufs=2))
                rpool = ph2.enter_context(tc.tile_pool(name="rpool", bufs=2))
                otpool = ph2.enter_context(tc.tile_pool(name="otpool", bufs=1))
                opool = ph2.enter_context(tc.tile_pool(name="opool", bufs=2))
                ps_s = ph2.enter_context(
                    tc.tile_pool(name="ps_s", bufs=3, space="PSUM"))
                ps_av = ph2.enter_context(
                    tc.tile_pool(name="ps_av", bufs=2, space="PSUM"))
                ps_lbc = ph2.enter_context(
                    tc.tile_pool(name="ps_lbc", bufs=1, space="PSUM"))
                ps_o = ph2.enter_context(
                    tc.tile_pool(name="ps_o", bufs=2, space="PSUM"))

                for b in range(B):
                    tb = b * T
                    outTs = {qc: otpool.tile([H, NHC, TCH], bf16,
                                             tag=f"outT{qc}",
                                             name=f"outT{qc}")
                             for qc in range(NTCH)}

                    def attn_stream(qc, h):
                        n_st = (qc + 1) * NSUB
                        rhs_q = qTs[b][qc][:, h, :]
                        av_ps = ps_av.tile([H, TCH], f32, tag="av",
                                           name="av_ps")
                        lsum = lpool.tile([H, TCH], bf16, tag="lsum",
                                          name="lsum")

                        def scores_block(st):
                            sps = ps_s.tile([H, TCH], f32, tag="s",
                                            name="sps")
                            kt = kTs[b][st // NSUB][
                                :, (st % NSUB) * H:(st % NSUB + 1) * H]
                            nc.tensor.matmul(sps, kt, rhs_q,
                                             start=True, stop=True)
                            j = st - qc * NSUB
                            pT2 = p2pool.tile([H, TCH], bf16, tag="p2",
                                              name="pT2")
                            if j >= 0:
                                pT = ppool.tile([H, TCH], bf16, tag="p",
                                                name="pT")
                                nc.scalar.activation(pT, sps, Exp,
                                                     scale=C_SM)
                                nc.vector.tensor_mul(pT2, pT, masks[j])
                            else:
                                nc.scalar.activation(pT2, sps, Exp,
                                                     scale=C_SM)
                            # softmax denominator accumulates on DVE, off
                            # the PE's critical path
                            if st == 0:
                                nc.vector.tensor_copy(lsum, pT2)
                            else:
                                nc.vector.tensor_add(lsum, lsum, pT2)
                            return pT2

                        def av_block(st, pT2):
                            nc.tensor.matmul(
                                av_ps, vs[b][st // NSUB][:, st % NSUB, :],
                                pT2,
                                start=(st == 0), stop=(st == n_st - 1),
                                skip_group_check=True)

                        # lookahead-2: two score blocks in flight ahead of
                        # each AV so the exp/mask latency never stalls the PE
                        pending = [scores_block(0), scores_block(1)]
                        for st in range(2, n_st):
                            pending.append(scores_block(st))
                            av_block(st - 2, pending.pop(0))
                        av_block(n_st - 2, pending.pop(0))
                        av_block(n_st - 1, pending.pop(0))
                        # partition-sum + broadcast of the denominator in one
                        # all-ones matmul, then fast reciprocal + normalize
                        lbc = ps_lbc.tile([H, TCH], f32, tag="lbc",
                                          name="lbc")
                        nc.tensor.matmul(lbc, ones128, lsum,
                                         start=True, stop=True)
                        rl = rpool.tile([H, TCH], f32, tag="rl", name="rl")
                        nc.vector.reciprocal_approx_fast(out=rl, in_=lbc)
                        nc.vector.tensor_mul(outTs[qc][:, h, :], av_ps, rl)

                    def emit_oproj(qc):
                        outT = outTs[qc]
                        for u in range(NSUB):
                            trow = tb + qc * TCH + u * H
                            for dh in range(2):
                                # 4 PSUM evictions batched into one 4KB-row
                                # store: keeps the Sync queue off the
                                # critical path
                                o_sb = opool.tile([H, 4, TCH], bf16,
                                                  tag="osb", name="o_sb")
                                for j in range(4):
                                    dc = dh * 4 + j
                                    ops = ps_o.tile([H, TCH], f32, tag="o",
                                                    name="ops")
                                    for h in range(NHC):
                                        nc.tensor.matmul(
                                            ops,
                                            outT[:, h, u * H:(u + 1) * H],
                                            wo_t[dc // 2][:, h,
                                                          (dc % 2) * TCH:
                                                          (dc % 2 + 1) * TCH],
                                            start=(h == 0),
                                            stop=(h == NHC - 1),
                                            skip_group_check=True)
                                    nc.scalar.activation(
                                        o_sb[:, j, :], ops, Copy)
                                nc.sync.dma_start(
                                    out=o_part[trow:trow + H,
                                               dh * 2048:(dh + 1) * 2048],
                                    in_=o_sb)

                    streams = []
                    for qa, qb in ((3, 0), (2, 1)):
                        for h in range(NHC):
                            streams.append((qa, h))
                            streams.append((qb, h))
                    for i, (qc, h) in enumerate(streams):
                        attn_stream(qc, h)
                        if i == 8:
                            emit_oproj(3)
                        elif i == 9:
                            emit_oproj(0)
                    emit_oproj(2)
                    emit_oproj(1)

    nc.compile()
    return nc


_NC_CACHE = None


def _prep_inputs(x, wq, wk, wv, wo, positions):
    import ml_dtypes
    bf = ml_dtypes.bfloat16

    x = np.asarray(x, dtype=np.float32)
    wq = np.asarray(wq, dtype=np.float32)
    wk = np.asarray(wk, dtype=np.float32)
    wv = np.asarray(wv, dtype=np.float32)
    wo = np.asarray(wo, dtype=np.float32)
    positions = np.asarray(positions)

    xT = np.ascontiguousarray(x.reshape(TOK, D).T.astype(bf))
    # rope tables [H, TOK], duplicated across halves, sin top half negated
    fraction = 2.0 * np.arange(HH, dtype=np.float32) / H
    timescale = (THETA ** fraction).astype(np.float32)
    pos = positions.reshape(TOK).astype(np.float32)
    sinu = pos[None, :] / timescale[:, None]
    cos = np.cos(sinu).astype(np.float32)
    sin = np.sin(sinu).astype(np.float32)
    cosT = np.ascontiguousarray(np.concatenate([cos, cos], 0).astype(bf))
    sinT = np.ascontiguousarray(np.concatenate([-sin, sin], 0).astype(bf))

    in_maps = []
    for c in range(NCORES):
        wq_c = wq[c * NHC:(c + 1) * NHC]            # [4, D, H]
        wqs = np.ascontiguousarray(
            wq_c.reshape(NHC, KPC, H, H).transpose(2, 0, 1, 3).astype(bf))
        wks = np.ascontiguousarray(
            wk[c].reshape(KPC, H, H).transpose(1, 0, 2).astype(bf))
        wvs = np.ascontiguousarray(
            wv[c].reshape(KPC, H, H).transpose(1, 0, 2).astype(bf))
        wos = np.ascontiguousarray(
            wo[c * NHC:(c + 1) * NHC].transpose(1, 0, 2).astype(bf))
        in_maps.append({
            "xT": xT,
            "wqs": wqs,
            "wks": wks,
            "wvs": wvs,
            "wos": wos,
            "cosT": cosT,
            "sinT": sinT,
        })
    return in_maps


def kernel(x, wq, wk, wv, wo, positions):
    global _NC_CACHE
    from concourse.bass_utils import run_bass_kernel_spmd

    in_maps = _prep_inputs(x, wq, wk, wv, wo, positions)

    if _NC_CACHE is None:
        _NC_CACHE = _build_bass()
    nc = _NC_CACHE

    trace = os.environ.get("BASS_KERNEL_TRACE", "0") == "1"
    res = run_bass_kernel_spmd(nc, in_maps, list(range(NCORES)), trace=trace)
    global LAST_RESULTS
    LAST_RESULTS = res
    out = np.zeros((TOK, D), dtype=np.float32)
    for c in range(NCORES):
        out += np.asarray(res.results[c]["o_part"]).astype(np.float32)
    return out.reshape(B, T, D)


LAST_RESULTS = None


# revision 14
# speedup vs baseline: 1.6193x; 1.0413x over previous
"""GQA causal-attention prefill kernel for Trainium2, tensor-parallel over 8 NeuronCores.

Reference semantics: q/k/v projections + RoPE + causal GQA attention +
output projection, B=2, T=2048, D=4096, 32 q heads, 8 kv heads, head_dim
128.  Core c owns q heads [4c, 4c+4), kv head c and the matching wo
slice; each core computes a full-shape partial output o_part and the
host sums the 8 partials (the tensor-parallel all-reduce).

Everything on the PE runs in bf16 (fp32 PSUM accumulation); measured
end-to-end max-rel error vs the fp32 reference is ~4e-3, well inside the
2e-2 gate, and bf16 halves DMA traffic, halves SBUF footprint (so all
weights + both batches' activations stay resident) and unlocks the
2-4x DVE 16-bit modes for the softmax bookkeeping.

Structure (emission order = engine program order):
  P1(b0), P1(b1):  projections + rope, TWO passes per batch over x
      (pass A: q0,q1,k; pass B: q2,q3,v).  3 accumulation groups x
      bufs=2 PSUM banks -> evictions of chunk c overlap the full 20us
      K-sweep of chunk c+1, so the PE never waits on a bank.  x is read
      twice (bf16 makes the 2x stream fit in HBM bandwidth); weights
      are loaded once up front, in k-group tiles so the first matmul
      only waits for ~1.5MB.
  P2(b0), P2(b1):  attention + o-projection per 512-token q-chunk.
      Scores transposed (sT = kT.T @ qT) so AV contracts s on the
      partition dim.  Softmax denominator comes from DVE adds of the
      exp tiles (off the PE) + ONE all-ones [128,128] matmul per
      (chunk, head) that sums over partitions AND broadcasts in one
      shot; 1/l via the fast custom-DVE reciprocal.  q-chunks are
      processed in pairs {3,0},{2,1} with heads interleaved so each
      stream's finalize chain hides behind a long stream's matmuls,
      and the o-projection of finished chunks is emitted between
      streams to keep the PE queue deep.
"""

import os
import sys

sys.path.insert(0, "/opt/trn_rl_repo")

import numpy as np

B = 2
T = 2048
TOK = B * T
D = 4096
NQ = 32
NKV = 8
H = 128
HH = H // 2
THETA = 10000.0
NCORES = 8
NHC = NQ // NCORES          # q heads per core (4)
KPC = D // H                # contraction chunks of 128 over D (32)
KG = 4                      # k-groups per weight tensor (8 chunks each)
TCH = 512                   # token chunk
NTCH = T // TCH             # 4 token chunks per batch
NSUB = TCH // H             # 4 s-subtiles per chunk
C_SM = 1.0 / np.sqrt(H)     # softmax scale


def _build_bass():
    import concourse.bacc as bacc
    import concourse.mybir as mybir
    import concourse.tile as tile
    from concourse.masks import make_identity
    from contextlib import ExitStack

    f32 = mybir.dt.float32
    bf16 = mybir.dt.bfloat16
    Exp = mybir.ActivationFunctionType.Exp
    Copy = mybir.ActivationFunctionType.Copy

    nc = bacc.Bacc("TRN2", target_bir_lowering=False, debug=False,
                   num_devices=NCORES)

    xT = nc.declare_dram_parameter("xT", [D, TOK], bf16, isOutput=False)
    # host pre-shuffled so every DMA row is >=2KB contiguous:
    # wqs[p, h, c, m] = wq[h, c*128+p, m]
    wqs = nc.declare_dram_parameter("wqs", [H, NHC, KPC, H], bf16,
                                    isOutput=False)
    wks = nc.declare_dram_parameter("wks", [H, KPC, H], bf16, isOutput=False)
    wvs = nc.declare_dram_parameter("wvs", [H, KPC, H], bf16, isOutput=False)
    # wos[p, h, d] = wo[h, p, d]
    wos = nc.declare_dram_parameter("wos", [H, NHC, D], bf16, isOutput=False)
    # rope tables duplicated across partition halves; sinT's TOP half is
    # NEGATED on the host so rope is out = direct*cosT + swap*sinT for all
    # 128 partitions in one mul+mul+add.
    cosT = nc.declare_dram_parameter("cosT", [H, TOK], bf16, isOutput=False)
    sinT = nc.declare_dram_parameter("sinT", [H, TOK], bf16, isOutput=False)
    o_part = nc.declare_dram_parameter("o_part", [TOK, D], bf16, isOutput=True)
    # x viewed as [p, kchunk, t] so one DMA start can fetch 4 k-chunks
    # (each dma_start costs ~600ns of serial Sync-sequencer time; the
    # un-batched version saturated that queue)
    xTv = xT.rearrange("(c p) t -> p c t", p=H)

    with tile.TileContext(nc) as tc:
        with ExitStack() as top:
            consts = top.enter_context(tc.tile_pool(name="consts", bufs=1))
            identity = consts.tile([H, H], bf16)
            make_identity(nc, identity)
            ones128 = consts.tile([H, H], bf16, tag="ones128")
            nc.vector.memset(ones128, 1.0)
            # 0/1 causal wedge masks: mask[j][s, t] = 1 iff (t - s - 128*j) >= 0
            masks = []
            for j in range(NSUB):
                m = consts.tile([H, TCH], bf16, tag=f"mask{j}",
                                name=f"mask{j}")
                nc.vector.memset(m, 1.0)
                nc.gpsimd.affine_select(
                    out=m, in_=m,
                    compare_op=mybir.AluOpType.is_ge,
                    fill=0.0,
                    base=-H * j,
                    pattern=[[1, TCH]],
                    channel_multiplier=-1,
                )
                masks.append(m)

            # ---- weights: loaded once, staged so x streaming stays ahead ----
            wpool = top.enter_context(tc.tile_pool(name="wpool", bufs=1))
            wq_t = [[wpool.tile([H, 8, H], bf16, tag=f"wq{h}_{g}",
                                name=f"wq{h}_{g}") for g in range(KG)]
                    for h in range(NHC)]
            wk_t = [wpool.tile([H, 8, H], bf16, tag=f"wk{g}", name=f"wk{g}")
                    for g in range(KG)]
            wv_t = [wpool.tile([H, 8, H], bf16, tag=f"wv{g}", name=f"wv{g}")
                    for g in range(KG)]
            wo_t = [wpool.tile([H, NHC, 1024], bf16, tag=f"wo{dq}",
                               name=f"wo{dq}") for dq in range(4)]
            cos_t = [wpool.tile([H, T], bf16, tag=f"cos{b}", name=f"cos{b}")
                     for b in range(B)]
            sin_t = [wpool.tile([H, T], bf16, tag=f"sin{b}", name=f"sin{b}")
                     for b in range(B)]

            # immediately needed: pass-A k-group 0; everything else is
            # drained between x loads so the first x tile isn't queued
            # behind megabytes of weights
            nc.sync.dma_start(out=wq_t[0][0], in_=wqs[:, 0, 0:8, :])
            nc.sync.dma_start(out=wq_t[1][0], in_=wqs[:, 1, 0:8, :])
            nc.sync.dma_start(out=wk_t[0], in_=wks[:, 0:8, :])
            pend = []
            for g in range(1, KG):
                pend.append((wq_t[0][g], wqs[:, 0, g * 8:(g + 1) * 8, :]))
                pend.append((wq_t[1][g], wqs[:, 1, g * 8:(g + 1) * 8, :]))
                pend.append((wk_t[g], wks[:, g * 8:(g + 1) * 8, :]))
            pend.append((cos_t[0], cosT[:, 0:T]))
            pend.append((sin_t[0], sinT[:, 0:T]))
            for g in range(KG):
                pend.append((wq_t[2][g], wqs[:, 2, g * 8:(g + 1) * 8, :]))
                pend.append((wq_t[3][g], wqs[:, 3, g * 8:(g + 1) * 8, :]))
                pend.append((wv_t[g], wvs[:, g * 8:(g + 1) * 8, :]))
            pend.append((cos_t[1], cosT[:, T:TOK]))
            pend.append((sin_t[1], sinT[:, T:TOK]))
            for dq in range(4):
                pend.append((wo_t[dq], wos[:, :, dq * 1024:(dq + 1) * 1024]))

            def drain_pend(n):
                for _ in range(n):
                    if pend:
                        dst, src = pend.pop(0)
                        nc.sync.dma_start(out=dst, in_=src)

            # ---- activations, both batches resident (bf16) ----
            apool = top.enter_context(tc.tile_pool(name="apool", bufs=1))
            qTs = [[apool.tile([H, NHC, TCH], bf16, tag=f"qT{b}_{i}",
                               name=f"qT{b}_{i}") for i in range(NTCH)]
                   for b in range(B)]
            kTs = [[apool.tile([H, TCH], bf16, tag=f"kT{b}_{i}",
                               name=f"kT{b}_{i}") for i in range(NTCH)]
                   for b in range(B)]
            vs = [[apool.tile([H, NSUB, H], bf16, tag=f"v{b}_{i}",
                              name=f"v{b}_{i}") for i in range(NTCH)]
                  for b in range(B)]

            # ================= phase 1: projections + rope =================
            with ExitStack() as ph1:
                xpool = ph1.enter_context(tc.tile_pool(name="xpool", bufs=4))
                rtmp = ph1.enter_context(tc.tile_pool(name="rtmp", bufs=2))
                pj = ph1.enter_context(
                    tc.tile_pool(name="pj", bufs=2, space="PSUM"))
                pt = ph1.enter_context(
                    tc.tile_pool(name="pt", bufs=2, space="PSUM"))

                def rope_from_psum(psum, dst_ap, cs, sn):
                    # swap staging: halves exchanged so the mul against the
                    # (half-duplicated) rope table is one full-width op.
                    swap = rtmp.tile([H, TCH], f32, tag="swap", bufs=3,
                                     name="swap")
                    nc.vector.tensor_copy(swap[0:HH, :], psum[HH:H, :])
                    nc.vector.tensor_copy(swap[HH:H, :], psum[0:HH, :])
                    m1 = rtmp.tile([H, TCH], f32, tag="m1", name="m1")
                    m2 = rtmp.tile([H, TCH], f32, tag="m2", name="m2")
                    nc.vector.tensor_mul(m1, psum, cs)
                    nc.vector.tensor_mul(m2, swap, sn)
                    nc.vector.tensor_add(dst_ap, m1, m2)

                # staged weight-DMA drain counts, interleaved between the
                # x loads of batch 0 (emission precedes every consumer —
                # Tile deps follow emission order; kg g's weights are
                # drained right before the x group that consumes them).
                drains = {0: [[0, 3, 3, 5], [4, 0, 0, 0], [4, 0, 0, 0],
                              [4, 0, 0, 0]],
                          1: [[2, 0, 0, 0], [2, 0, 0, 0], [2, 0, 0, 0],
                              [0, 0, 0, 0]]}
                for b in range(B):
                    tb = b * T
                    for pas in range(2):
                        for tch in range(NTCH):
                            t0 = tch * TCH
                            g_ps = [pj.tile([H, TCH], f32, tag=f"g{i}",
                                            name=f"g{i}") for i in range(3)]
                            for kq in range(KPC // 8):
                                if b == 0:
                                    drain_pend(drains[pas][tch][kq])
                                x_t = xpool.tile([H, 8, TCH], bf16, tag="x",
                                                 name="x_t")
                                nc.sync.dma_start(
                                    out=x_t,
                                    in_=xTv[:, kq * 8:(kq + 1) * 8,
                                            tb + t0:tb + t0 + TCH])
                                for kc in range(8):
                                    k = kq * 8 + kc
                                    if pas == 0:
                                        lhs = [wq_t[0][kq][:, kc, :],
                                               wq_t[1][kq][:, kc, :],
                                               wk_t[kq][:, kc, :]]
                                    else:
                                        lhs = [wq_t[2][kq][:, kc, :],
                                               wq_t[3][kq][:, kc, :],
                                               wv_t[kq][:, kc, :]]
                                    for gi in range(3):
                                        nc.tensor.matmul(
                                            g_ps[gi], lhs[gi], x_t[:, kc, :],
                                            start=(k == 0),
                                            stop=(k == KPC - 1),
                                            skip_group_check=True)
                            cs = cos_t[b][:, t0:t0 + TCH]
                            sn = sin_t[b][:, t0:t0 + TCH]
                            if pas == 0:
                                rope_from_psum(g_ps[2], kTs[b][tch], cs, sn)
                                rope_from_psum(g_ps[0], qTs[b][tch][:, 0, :],
                                               cs, sn)
                                rope_from_psum(g_ps[1], qTs[b][tch][:, 1, :],
                                               cs, sn)
                            else:
                                vstage = rtmp.tile([H, TCH], bf16,
                                                   tag="vstage", name="vstage")
                                nc.vector.tensor_copy(vstage, g_ps[2])
                                for j in range(NSUB):
                                    tp = pt.tile([H, H], bf16, tag="vtp",
                                                 name="vtp")
                                    nc.tensor.transpose(
                                        tp, vstage[:, j * H:(j + 1) * H],
                                        identity)
                                    nc.vector.tensor_copy(
                                        vs[b][tch][:, j, :], tp)
                                rope_from_psum(g_ps[0], qTs[b][tch][:, 2, :],
                                               cs, sn)
                                rope_from_psum(g_ps[1], qTs[b][tch][:, 3, :],
                                               cs, sn)

            # ============= phase 2: attention + o-projection =============
            with ExitStack() as ph2:
                ppool = ph2.enter_context(tc.tile_pool(name="ppool", bufs=2))
                p2pool = ph2.enter_context(tc.tile_pool(name="p2pool", bufs=4))
                lpool = ph2.enter_context(tc.tile_pool(name="lpool", bufs=2))
                rpool = ph2.enter_context(tc.tile_pool(name="rpool", bufs=2))
                otpool = ph2.enter_context(tc.tile_pool(name="otpool", bufs=1))
                opool = ph2.enter_context(tc.tile_pool(name="opool", bufs=2))
                ps_s = ph2.enter_context(
                    tc.tile_pool(name="ps_s", bufs=3, space="PSUM"))
                ps_av = ph2.enter_context(
                    tc.tile_pool(name="ps_av", bufs=2, space="PSUM"))
                ps_lbc = ph2.enter_context(
                    tc.tile_pool(name="ps_lbc", bufs=1, space="PSUM"))
                ps_o = ph2.enter_context(
                    tc.tile_pool(name="ps_o", bufs=2, space="PSUM"))

                def attn_stream(b, outTs, qc, h, fin_prev):
                    """Emit one (q-chunk, head) stream's matmuls; return a
                    finalize thunk (denominator broadcast + reciprocal +
                    normalize) that the NEXT stream runs after its first
                    score block, so the PE never waits on the DVE lsum
                    tail."""
                    n_st = (qc + 1) * NSUB
                    rhs_q = qTs[b][qc][:, h, :]
                    av_ps = ps_av.tile([H, TCH], f32, tag="av",
                                       name="av_ps")
                    lsum = lpool.tile([H, TCH], bf16, tag="lsum",
                                      name="lsum")

                    def scores_block(st):
                        sps = ps_s.tile([H, TCH], f32, tag="s", name="sps")
                        kt = kTs[b][st // NSUB][
                            :, (st % NSUB) * H:(st % NSUB + 1) * H]
                        nc.tensor.matmul(sps, kt, rhs_q,
                                         start=True, stop=True)
                        j = st - qc * NSUB
                        pT2 = p2pool.tile([H, TCH], bf16, tag="p2",
                                          name="pT2")
                        if j >= 0:
                            pT = ppool.tile([H, TCH], bf16, tag="p",
                                            name="pT")
                            nc.scalar.activation(pT, sps, Exp, scale=C_SM)
                            nc.vector.tensor_mul(pT2, pT, masks[j])
                        else:
                            nc.scalar.activation(pT2, sps, Exp, scale=C_SM)
                        # softmax denominator accumulates on DVE, off the
                        # PE's critical path
                        if st == 0:
                            nc.vector.tensor_copy(lsum, pT2)
                        else:
                            nc.vector.tensor_add(lsum, lsum, pT2)
                        return pT2

                    def av_block(st, pT2):
                        nc.tensor.matmul(
                            av_ps, vs[b][st // NSUB][:, st % NSUB, :], pT2,
                            start=(st == 0), stop=(st == n_st - 1),
                            skip_group_check=True)

                    # lookahead-2: two score blocks in flight ahead of each
                    # AV so the exp/mask latency never stalls the PE
                    pending = [scores_block(0)]
                    if fin_prev is not None:
                        fin_prev()
                    pending.append(scores_block(1))
                    for st in range(2, n_st):
                        pending.append(scores_block(st))
                        av_block(st - 2, pending.pop(0))
                    av_block(n_st - 2, pending.pop(0))
                    av_block(n_st - 1, pending.pop(0))

                    def fin():
                        # partition-sum + broadcast of the denominator in
                        # one all-ones matmul, then fast reciprocal +
                        # normalize
                        lbc = ps_lbc.tile([H, TCH], f32, tag="lbc",
                                          name="lbc")
                        nc.tensor.matmul(lbc, ones128, lsum,
                                         start=True, stop=True)
                        rl = rpool.tile([H, TCH], f32, tag="rl", name="rl")
                        nc.vector.reciprocal_approx_fast(out=rl, in_=lbc)
                        nc.vector.tensor_mul(outTs[qc][:, h, :], av_ps, rl)

                    return fin

                def oproj_units(b, outTs, qc):
                    """o-projection of one q-chunk as 8 independent thunks
                    (one per (u, dh)), drained between attention streams to
                    keep the PE queue deep."""
                    tb = b * T
                    outT = outTs[qc]
                    units = []
                    for u in range(NSUB):
                        for dh in range(2):
                            def unit(u=u, dh=dh):
                                trow = tb + qc * TCH + u * H
                                # 4 PSUM evictions batched into one 4KB-row
                                # store: keeps the Sync queue off the
                                # critical path
                                o_sb = opool.tile([H, 4, TCH], bf16,
                                                  tag="osb", name="o_sb")
                                for j in range(4):
                                    dc = dh * 4 + j
                                    ops = ps_o.tile([H, TCH], f32, tag="o",
                                                    name="ops")
                                    for h in range(NHC):
                                        nc.tensor.matmul(
                                            ops,
                                            outT[:, h, u * H:(u + 1) * H],
                                            wo_t[dc // 2][:, h,
                                                          (dc % 2) * TCH:
                                                          (dc % 2 + 1) * TCH],
                                            start=(h == 0),
                                            stop=(h == NHC - 1),
                                            skip_group_check=True)
                                    nc.scalar.activation(
                                        o_sb[:, j, :], ops, Copy)
                                nc.sync.dma_start(
                                    out=o_part[trow:trow + H,
                                               dh * 2048:(dh + 1) * 2048],
                                    in_=o_sb)
                            units.append(unit)
                    return units

                opq = []        # pending o-proj units, carried across batches
                fin = None
                for b in range(B):
                    outTs = {qc: otpool.tile([H, NHC, TCH], bf16,
                                             tag=f"outT{qc}",
                                             name=f"outT{qc}")
                             for qc in range(NTCH)}
                    streams = []
                    for qa, qb in ((3, 0), (2, 1)):
                        for h in range(NHC):
                            streams.append((qa, h))
                            streams.append((qb, h))
                    for i, (qc, h) in enumerate(streams):
                        fin = attn_stream(b, outTs, qc, h, fin)
                        if i == 8:
                            opq += oproj_units(b, outTs, 3)
                        elif i == 9:
                            opq += oproj_units(b, outTs, 0)
                        for _ in range(2):
                            if opq:
                                opq.pop(0)()
                    opq += oproj_units(b, outTs, 2)
                    opq += oproj_units(b, outTs, 1)
                fin()
                for unit in opq:
                    unit()

    nc.compile()
    return nc


_NC_CACHE = None


def _prep_inputs(x, wq, wk, wv, wo, positions):
    import ml_dtypes
    bf = ml_dtypes.bfloat16

    x = np.asarray(x, dtype=np.float32)
    wq = np.asarray(wq, dtype=np.float32)
    wk = np.asarray(wk, dtype=np.float32)
    wv = np.asarray(wv, dtype=np.float32)
    wo = np.asarray(wo, dtype=np.float32)
    positions = np.asarray(positions)

    xT = np.ascontiguousarray(x.reshape(TOK, D).T.astype(bf))
    # rope tables [H, TOK], duplicated across halves, sin top half negated
    fraction = 2.0 * np.arange(HH, dtype=np.float32) / H
    timescale = (THETA ** fraction).astype(np.float32)
    pos = positions.reshape(TOK).astype(np.float32)
    sinu = pos[None, :] / timescale[:, None]
    cos = np.cos(sinu).astype(np.float32)
    sin = np.sin(sinu).astype(np.float32)
    cosT = np.ascontiguousarray(np.concatenate([cos, cos], 0).astype(bf))
    sinT = np.ascontiguousarray(np.concatenate([-sin, sin], 0).astype(bf))

    in_maps = []
    for c in range(NCORES):
        wq_c = wq[c * NHC:(c + 1) * NHC]            # [4, D, H]
        wqs = np.ascontiguousarray(
            wq_c.reshape(NHC, KPC, H, H).transpose(2, 0, 1, 3).astype(bf))
        wks = np.ascontiguousarray(
            wk[c].reshape(KPC, H, H).transpose(1, 0, 2).astype(bf))
        wvs = np.ascontiguousarray(
            wv[c].reshape(KPC, H, H).transpose(1, 0, 2).astype(bf))
        wos = np.ascontiguousarray(
            wo[c * NHC:(c + 1) * NHC].transpose(1, 0, 2).astype(bf))
        in_maps.append({
            "xT": xT,
            "wqs": wqs,
            "wks": wks,
            "wvs": wvs,
            "wos": wos,
            "cosT": cosT,
            "sinT": sinT,
        })
    return in_maps


def kernel(x, wq, wk, wv, wo, positions):
    global _NC_CACHE
    from concourse.bass_utils import run_bass_kernel_spmd

    in_maps = _prep_inputs(x, wq, wk, wv, wo, positions)

    if _NC_CACHE is None:
        _NC_CACHE = _build_bass()
    nc = _NC_CACHE

    trace = os.environ.get("BASS_KERNEL_TRACE", "0") == "1"
    res = run_bass_kernel_spmd(nc, in_maps, list(range(NCORES)), trace=trace)
    global LAST_RESULTS
    LAST_RESULTS = res
    out = np.zeros((TOK, D), dtype=np.float32)
    for c in range(NCORES):
        out += np.asarray(res.results[c]["o_part"]).astype(np.float32)
    return out.reshape(B, T, D)


LAST_RESULTS = None


# revision 16
# speedup vs baseline: 1.6231x; 1.0024x over previous
"""GQA causal-attention prefill kernel for Trainium2, tensor-parallel over 8 NeuronCores.

Reference semantics: q/k/v projections + RoPE + causal GQA attention +
output projection, B=2, T=2048, D=4096, 32 q heads, 8 kv heads, head_dim
128.  Core c owns q heads [4c, 4c+4), kv head c and the matching wo
slice; each core computes a full-shape partial output o_part and the
host sums the 8 partials (the tensor-parallel all-reduce).

Everything on the PE runs in bf16 (fp32 PSUM accumulation); measured
end-to-end max-rel error vs the fp32 reference is ~4e-3, well inside the
2e-2 gate, and bf16 halves DMA traffic, halves SBUF footprint (so all
weights + both batches' activations stay resident) and unlocks the
2-4x DVE 16-bit modes for the softmax bookkeeping.

Structure (emission order = engine program order):
  P1(b0), P1(b1):  projections + rope, TWO passes per batch over x
      (pass A: q0,q1,k; pass B: q2,q3,v).  3 accumulation groups x
      bufs=2 PSUM banks -> evictions of chunk c overlap the full 20us
      K-sweep of chunk c+1, so the PE never waits on a bank.  x is read
      twice (bf16 makes the 2x stream fit in HBM bandwidth); weights
      are loaded once up front, in k-group tiles so the first matmul
      only waits for ~1.5MB.
  P2(b0), P2(b1):  attention + o-projection per 512-token q-chunk.
      Scores transposed (sT = kT.T @ qT) so AV contracts s on the
      partition dim.  Softmax denominator comes from DVE adds of the
      exp tiles (off the PE) + ONE all-ones [128,128] matmul per
      (chunk, head) that sums over partitions AND broadcasts in one
      shot; 1/l via the fast custom-DVE reciprocal.  q-chunks are
      processed in pairs {3,0},{2,1} with heads interleaved so each
      stream's finalize chain hides behind a long stream's matmuls,
      and the o-projection of finished chunks is emitted between
      streams to keep the PE queue deep.
"""

import os
import sys

sys.path.insert(0, "/opt/trn_rl_repo")

import numpy as np

B = 2
T = 2048
TOK = B * T
D = 4096
NQ = 32
NKV = 8
H = 128
HH = H // 2
THETA = 10000.0
NCORES = 8
NHC = NQ // NCORES          # q heads per core (4)
KPC = D // H                # contraction chunks of 128 over D (32)
KG = 4                      # k-groups per weight tensor (8 chunks each)
TCH = 512                   # token chunk
NTCH = T // TCH             # 4 token chunks per batch
NSUB = TCH // H             # 4 s-subtiles per chunk
C_SM = 1.0 / np.sqrt(H)     # softmax scale


def _build_bass():
    import concourse.bacc as bacc
    import concourse.mybir as mybir
    import concourse.tile as tile
    from concourse.masks import make_identity
    from contextlib import ExitStack

    f32 = mybir.dt.float32
    bf16 = mybir.dt.bfloat16
    Exp = mybir.ActivationFunctionType.Exp
    Copy = mybir.ActivationFunctionType.Copy

    nc = bacc.Bacc("TRN2", target_bir_lowering=False, debug=False,
                   num_devices=NCORES)

    xT = nc.declare_dram_parameter("xT", [D, TOK], bf16, isOutput=False)
    # host pre-shuffled so every DMA row is >=2KB contiguous:
    # wqs[p, h, c, m] = wq[h, c*128+p, m]
    wqs = nc.declare_dram_parameter("wqs", [H, NHC, KPC, H], bf16,
                                    isOutput=False)
    wks = nc.declare_dram_parameter("wks", [H, KPC, H], bf16, isOutput=False)
    wvs = nc.declare_dram_parameter("wvs", [H, KPC, H], bf16, isOutput=False)
    # wos[p, h, d] = wo[h, p, d]
    wos = nc.declare_dram_parameter("wos", [H, NHC, D], bf16, isOutput=False)
    # rope tables duplicated across partition halves; sinT's TOP half is
    # NEGATED on the host so rope is out = direct*cosT + swap*sinT for all
    # 128 partitions in one mul+mul+add.
    cosT = nc.declare_dram_parameter("cosT", [H, TOK], bf16, isOutput=False)
    sinT = nc.declare_dram_parameter("sinT", [H, TOK], bf16, isOutput=False)
    o_part = nc.declare_dram_parameter("o_part", [TOK, D], bf16, isOutput=True)
    # x viewed as [p, kchunk, t] so one DMA start can fetch 4 k-chunks
    # (each dma_start costs ~600ns of serial Sync-sequencer time; the
    # un-batched version saturated that queue)
    xTv = xT.rearrange("(c p) t -> p c t", p=H)

    with tile.TileContext(nc) as tc:
        with ExitStack() as top:
            consts = top.enter_context(tc.tile_pool(name="consts", bufs=1))
            identity = consts.tile([H, H], bf16)
            make_identity(nc, identity)
            ones128 = consts.tile([H, H], bf16, tag="ones128")
            nc.vector.memset(ones128, 1.0)
            # 0/1 causal wedge masks: mask[j][s, t] = 1 iff (t - s - 128*j) >= 0
            masks = []
            for j in range(NSUB):
                m = consts.tile([H, TCH], bf16, tag=f"mask{j}",
                                name=f"mask{j}")
                nc.vector.memset(m, 1.0)
                nc.gpsimd.affine_select(
                    out=m, in_=m,
                    compare_op=mybir.AluOpType.is_ge,
                    fill=0.0,
                    base=-H * j,
                    pattern=[[1, TCH]],
                    channel_multiplier=-1,
                )
                masks.append(m)

            # ---- weights: loaded once, staged so x streaming stays ahead ----
            wpool = top.enter_context(tc.tile_pool(name="wpool", bufs=1))
            wq_t = [[wpool.tile([H, 8, H], bf16, tag=f"wq{h}_{g}",
                                name=f"wq{h}_{g}") for g in range(KG)]
                    for h in range(NHC)]
            wk_t = [wpool.tile([H, 8, H], bf16, tag=f"wk{g}", name=f"wk{g}")
                    for g in range(KG)]
            wv_t = [wpool.tile([H, 8, H], bf16, tag=f"wv{g}", name=f"wv{g}")
                    for g in range(KG)]
            wo_t = [wpool.tile([H, NHC, 1024], bf16, tag=f"wo{dq}",
                               name=f"wo{dq}") for dq in range(4)]
            cos_t = [wpool.tile([H, T], bf16, tag=f"cos{b}", name=f"cos{b}")
                     for b in range(B)]
            sin_t = [wpool.tile([H, T], bf16, tag=f"sin{b}", name=f"sin{b}")
                     for b in range(B)]

            # immediately needed: pass-A k-group 0; everything else is
            # drained between x loads so the first x tile isn't queued
            # behind megabytes of weights
            nc.sync.dma_start(out=wq_t[0][0], in_=wqs[:, 0, 0:8, :])
            nc.sync.dma_start(out=wq_t[1][0], in_=wqs[:, 1, 0:8, :])
            nc.sync.dma_start(out=wk_t[0], in_=wks[:, 0:8, :])
            pend = []
            for g in range(1, KG):
                pend.append((wq_t[0][g], wqs[:, 0, g * 8:(g + 1) * 8, :]))
                pend.append((wq_t[1][g], wqs[:, 1, g * 8:(g + 1) * 8, :]))
                pend.append((wk_t[g], wks[:, g * 8:(g + 1) * 8, :]))
            pend.append((cos_t[0], cosT[:, 0:T]))
            pend.append((sin_t[0], sinT[:, 0:T]))
            for g in range(KG):
                pend.append((wq_t[2][g], wqs[:, 2, g * 8:(g + 1) * 8, :]))
                pend.append((wq_t[3][g], wqs[:, 3, g * 8:(g + 1) * 8, :]))
                pend.append((wv_t[g], wvs[:, g * 8:(g + 1) * 8, :]))
            pend.append((cos_t[1], cosT[:, T:TOK]))
            pend.append((sin_t[1], sinT[:, T:TOK]))
            for dq in range(4):
                pend.append((wo_t[dq], wos[:, :, dq * 1024:(dq + 1) * 1024]))

            def drain_pend(n):
                for _ in range(n):
                    if pend:
                        dst, src = pend.pop(0)
                        nc.sync.dma_start(out=dst, in_=src)

            # ---- activations, both batches resident (bf16) ----
            apool = top.enter_context(tc.tile_pool(name="apool", bufs=1))
            qTs = [[apool.tile([H, NHC, TCH], bf16, tag=f"qT{b}_{i}",
                               name=f"qT{b}_{i}") for i in range(NTCH)]
                   for b in range(B)]
            kTs = [[apool.tile([H, TCH], bf16, tag=f"kT{b}_{i}",
                               name=f"kT{b}_{i}") for i in range(NTCH)]
                   for b in range(B)]
            vs = [[apool.tile([H, NSUB, H], bf16, tag=f"v{b}_{i}",
                              name=f"v{b}_{i}") for i in range(NTCH)]
                  for b in range(B)]

            # ================= phase 1: projections + rope =================
            with ExitStack() as ph1:
                xpool = ph1.enter_context(tc.tile_pool(name="xpool", bufs=5))
                rtmp = ph1.enter_context(tc.tile_pool(name="rtmp", bufs=2))
                pj = ph1.enter_context(
                    tc.tile_pool(name="pj", bufs=2, space="PSUM"))
                pt = ph1.enter_context(
                    tc.tile_pool(name="pt", bufs=2, space="PSUM"))

                def rope_from_psum(psum, dst_ap, cs, sn):
                    # swap staging: halves exchanged so the mul against the
                    # (half-duplicated) rope table is one full-width op.
                    swap = rtmp.tile([H, TCH], f32, tag="swap", bufs=3,
                                     name="swap")
                    nc.vector.tensor_copy(swap[0:HH, :], psum[HH:H, :])
                    nc.vector.tensor_copy(swap[HH:H, :], psum[0:HH, :])
                    m1 = rtmp.tile([H, TCH], f32, tag="m1", name="m1")
                    m2 = rtmp.tile([H, TCH], f32, tag="m2", name="m2")
                    nc.vector.tensor_mul(m1, psum, cs)
                    nc.vector.tensor_mul(m2, swap, sn)
                    nc.vector.tensor_add(dst_ap, m1, m2)

                # staged weight-DMA drain counts, interleaved between the
                # x loads of batch 0 (emission precedes every consumer —
                # Tile deps follow emission order; kg g's weights are
                # drained right before the x group that consumes them).
                drains = {0: [[0, 3, 3, 5], [0, 4, 0, 0], [0, 4, 0, 0],
                              [0, 4, 0, 0]],
                          1: [[0, 2, 0, 0], [0, 2, 0, 0], [0, 2, 0, 0],
                              [0, 0, 0, 0]]}
                for b in range(B):
                    tb = b * T
                    for pas in range(2):
                        for tch in range(NTCH):
                            t0 = tch * TCH
                            g_ps = [pj.tile([H, TCH], f32, tag=f"g{i}",
                                            name=f"g{i}") for i in range(3)]
                            for kq in range(KPC // 8):
                                if b == 0:
                                    drain_pend(drains[pas][tch][kq])
                                x_t = xpool.tile([H, 8, TCH], bf16, tag="x",
                                                 name="x_t")
                                nc.sync.dma_start(
                                    out=x_t,
                                    in_=xTv[:, kq * 8:(kq + 1) * 8,
                                            tb + t0:tb + t0 + TCH])
                                for kc in range(8):
                                    k = kq * 8 + kc
                                    if pas == 0:
                                        lhs = [wq_t[0][kq][:, kc, :],
                                               wq_t[1][kq][:, kc, :],
                                               wk_t[kq][:, kc, :]]
                                    else:
                                        lhs = [wq_t[2][kq][:, kc, :],
                                               wq_t[3][kq][:, kc, :],
                                               wv_t[kq][:, kc, :]]
                                    for gi in range(3):
                                        nc.tensor.matmul(
                                            g_ps[gi], lhs[gi], x_t[:, kc, :],
                                            start=(k == 0),
                                            stop=(k == KPC - 1),
                                            skip_group_check=True)
                            cs = cos_t[b][:, t0:t0 + TCH]
                            sn = sin_t[b][:, t0:t0 + TCH]
                            if pas == 0:
                                rope_from_psum(g_ps[2], kTs[b][tch], cs, sn)
                                rope_from_psum(g_ps[0], qTs[b][tch][:, 0, :],
                                               cs, sn)
                                rope_from_psum(g_ps[1], qTs[b][tch][:, 1, :],
                                               cs, sn)
                            else:
                                vstage = rtmp.tile([H, TCH], bf16,
                                                   tag="vstage", name="vstage")
                                nc.vector.tensor_copy(vstage, g_ps[2])
                                for j in range(NSUB):
                                    tp = pt.tile([H, H], bf16, tag="vtp",
                                                 name="vtp")
                                    nc.tensor.transpose(
                                        tp, vstage[:, j * H:(j + 1) * H],
                                        identity)
                                    nc.vector.tensor_copy(
                                        vs[b][tch][:, j, :], tp)
                                rope_from_psum(g_ps[0], qTs[b][tch][:, 2, :],
                                               cs, sn)
                                rope_from_psum(g_ps[1], qTs[b][tch][:, 3, :],
                                               cs, sn)

            # ============= phase 2: attention + o-projection =============
            with ExitStack() as ph2:
                ppool = ph2.enter_context(tc.tile_pool(name="ppool", bufs=2))
                p2pool = ph2.enter_context(tc.tile_pool(name="p2pool", bufs=4))
                lpool = ph2.enter_context(tc.tile_pool(name="lpool", bufs=2))
                rpool = ph2.enter_context(tc.tile_pool(name="rpool", bufs=2))
                otpool = ph2.enter_context(tc.tile_pool(name="otpool", bufs=1))
                opool = ph2.enter_context(tc.tile_pool(name="opool", bufs=2))
                ps_s = ph2.enter_context(
                    tc.tile_pool(name="ps_s", bufs=3, space="PSUM"))
                ps_av = ph2.enter_context(
                    tc.tile_pool(name="ps_av", bufs=2, space="PSUM"))
                ps_lbc = ph2.enter_context(
                    tc.tile_pool(name="ps_lbc", bufs=1, space="PSUM"))
                ps_o = ph2.enter_context(
                    tc.tile_pool(name="ps_o", bufs=2, space="PSUM"))

                def attn_stream(b, outTs, qc, h, fin_prev):
                    """Emit one (q-chunk, head) stream's matmuls; return a
                    finalize thunk (denominator broadcast + reciprocal +
                    normalize) that the NEXT stream runs after its first
                    score block, so the PE never waits on the DVE lsum
                    tail."""
                    n_st = (qc + 1) * NSUB
                    rhs_q = qTs[b][qc][:, h, :]
                    av_ps = ps_av.tile([H, TCH], f32, tag="av",
                                       name="av_ps")
                    lsum = lpool.tile([H, TCH], bf16, tag="lsum",
                                      name="lsum")

                    def scores_block(st):
                        sps = ps_s.tile([H, TCH], f32, tag="s", name="sps")
                        kt = kTs[b][st // NSUB][
                            :, (st % NSUB) * H:(st % NSUB + 1) * H]
                        nc.tensor.matmul(sps, kt, rhs_q,
                                         start=True, stop=True)
                        j = st - qc * NSUB
                        pT2 = p2pool.tile([H, TCH], bf16, tag="p2",
                                          name="pT2")
                        if j >= 0:
                            pT = ppool.tile([H, TCH], bf16, tag="p",
                                            name="pT")
                            nc.scalar.activation(pT, sps, Exp, scale=C_SM)
                            nc.vector.tensor_mul(pT2, pT, masks[j])
                        else:
                            nc.scalar.activation(pT2, sps, Exp, scale=C_SM)
                        # softmax denominator accumulates on DVE, off the
                        # PE's critical path
                        if st == 0:
                            nc.vector.tensor_copy(lsum, pT2)
                        else:
                            nc.vector.tensor_add(lsum, lsum, pT2)
                        return pT2

                    def av_block(st, pT2):
                        nc.tensor.matmul(
                            av_ps, vs[b][st // NSUB][:, st % NSUB, :], pT2,
                            start=(st == 0), stop=(st == n_st - 1),
                            skip_group_check=True)

                    # lookahead-2: two score blocks in flight ahead of each
                    # AV so the exp/mask latency never stalls the PE
                    pending = [scores_block(0)]
                    if fin_prev is not None:
                        fin_prev()
                    pending.append(scores_block(1))
                    for st in range(2, n_st):
                        pending.append(scores_block(st))
                        av_block(st - 2, pending.pop(0))
                    av_block(n_st - 2, pending.pop(0))
                    av_block(n_st - 1, pending.pop(0))

                    def fin():
                        # partition-sum + broadcast of the denominator in
                        # one all-ones matmul, then fast reciprocal +
                        # normalize
                        lbc = ps_lbc.tile([H, TCH], f32, tag="lbc",
                                          name="lbc")
                        nc.tensor.matmul(lbc, ones128, lsum,
                                         start=True, stop=True)
                        rl = rpool.tile([H, TCH], f32, tag="rl", name="rl")
                        nc.vector.reciprocal_approx_fast(out=rl, in_=lbc)
                        nc.vector.tensor_mul(outTs[qc][:, h, :], av_ps, rl)

                    return fin

                def oproj_units(b, outTs, qc):
                    """o-projection of one q-chunk as 8 independent thunks
                    (one per (u, dh)), drained between attention streams to
                    keep the PE queue deep."""
                    tb = b * T
                    outT = outTs[qc]
                    units = []
                    for u in range(NSUB):
                        for dh in range(2):
                            def unit(u=u, dh=dh):
                                trow = tb + qc * TCH + u * H
                                # 4 PSUM evictions batched into one 4KB-row
                                # store: keeps the Sync queue off the
                                # critical path
                                o_sb = opool.tile([H, 4, TCH], bf16,
                                                  tag="osb", name="o_sb")
                                for j in range(4):
                                    dc = dh * 4 + j
                                    ops = ps_o.tile([H, TCH], f32, tag="o",
                                                    name="ops")
                                    for h in range(NHC):
                                        nc.tensor.matmul(
                                            ops,
                                            outT[:, h, u * H:(u + 1) * H],
                                            wo_t[dc // 2][:, h,
                                                          (dc % 2) * TCH:
                                                          (dc % 2 + 1) * TCH],
                                            start=(h == 0),
                                            stop=(h == NHC - 1),
                                            skip_group_check=True)
                                    nc.scalar.activation(
                                        o_sb[:, j, :], ops, Copy)
                                nc.sync.dma_start(
                                    out=o_part[trow:trow + H,
                                               dh * 2048:(dh + 1) * 2048],
                                    in_=o_sb)
                            units.append(unit)
                    return units

                opq = []        # pending o-proj units, carried across batches
                fin = None
                for b in range(B):
                    outTs = {qc: otpool.tile([H, NHC, TCH], bf16,
                                             tag=f"outT{qc}",
                                             name=f"outT{qc}")
                             for qc in range(NTCH)}
                    streams = []
                    for qa, qb in ((3, 0), (2, 1)):
                        for h in range(NHC):
                            streams.append((qa, h))
                            streams.append((qb, h))
                    for i, (qc, h) in enumerate(streams):
                        fin = attn_stream(b, outTs, qc, h, fin)
                        if i == 8:
                            opq += oproj_units(b, outTs, 3)
                        elif i == 9:
                            opq += oproj_units(b, outTs, 0)
                        for _ in range(2):
                            if opq:
                                opq.pop(0)()
                    opq += oproj_units(b, outTs, 2)
                    opq += oproj_units(b, outTs, 1)
                fin()
                for unit in opq:
                    unit()

    nc.compile()
    return nc


_NC_CACHE = None


def _prep_inputs(x, wq, wk, wv, wo, positions):
    import ml_dtypes
    bf = ml_dtypes.bfloat16

    x = np.asarray(x, dtype=np.float32)
    wq = np.asarray(wq, dtype=np.float32)
    wk = np.asarray(wk, dtype=np.float32)
    wv = np.asarray(wv, dtype=np.float32)
    wo = np.asarray(wo, dtype=np.float32)
    positions = np.asarray(positions)

    xT = np.ascontiguousarray(x.reshape(TOK, D).T.astype(bf))
    # rope tables [H, TOK], duplicated across halves, sin top half negated
    fraction = 2.0 * np.arange(HH, dtype=np.float32) / H
    timescale = (THETA ** fraction).astype(np.float32)
    pos = positions.reshape(TOK).astype(np.float32)
    sinu = pos[None, :] / timescale[:, None]
    cos = np.cos(sinu).astype(np.float32)
    sin = np.sin(sinu).astype(np.float32)
    cosT = np.ascontiguousarray(np.concatenate([cos, cos], 0).astype(bf))
    sinT = np.ascontiguousarray(np.concatenate([-sin, sin], 0).astype(bf))

    in_maps = []
    for c in range(NCORES):
        wq_c = wq[c * NHC:(c + 1) * NHC]            # [4, D, H]
        wqs = np.ascontiguousarray(
            wq_c.reshape(NHC, KPC, H, H).transpose(2, 0, 1, 3).astype(bf))
        wks = np.ascontiguousarray(
            wk[c].reshape(KPC, H, H).transpose(1, 0, 2).astype(bf))
        wvs = np.ascontiguousarray(
            wv[c].reshape(KPC, H, H).transpose(1, 0, 2).astype(bf))
        wos = np.ascontiguousarray(
            wo[c * NHC:(c + 1) * NHC].transpose(1, 0, 2).astype(bf))
        in_maps.append({
            "xT": xT,
            "wqs": wqs,
            "wks": wks,
            "wvs": wvs,
            "wos": wos,
            "cosT": cosT,
            "sinT": sinT,
        })
    return in_maps


def kernel(x, wq, wk, wv, wo, positions):
    global _NC_CACHE
    from concourse.bass_utils import run_bass_kernel_spmd

    in_maps = _prep_inputs(x, wq, wk, wv, wo, positions)

    if _NC_CACHE is None:
        _NC_CACHE = _build_bass()
    nc = _NC_CACHE

    trace = os.environ.get("BASS_KERNEL_TRACE", "0") == "1"
    res = run_bass_kernel_spmd(nc, in_maps, list(range(NCORES)), trace=trace)
    global LAST_RESULTS
    LAST_RESULTS = res
    out = np.zeros((TOK, D), dtype=np.float32)
    for c in range(NCORES):
        out += np.asarray(res.results[c]["o_part"]).astype(np.float32)
    return out.reshape(B, T, D)


LAST_RESULTS = None


# revision 20
# speedup vs baseline: 1.6677x; 1.0275x over previous
"""GQA causal-attention prefill kernel for Trainium2, tensor-parallel over 8 NeuronCores.

Reference semantics: q/k/v projections + RoPE + causal GQA attention +
output projection, B=2, T=2048, D=4096, 32 q heads, 8 kv heads, head_dim
128.  Core c owns q heads [4c, 4c+4), kv head c and the matching wo
slice; each core computes a full-shape partial output o_part and the
host sums the 8 partials (the tensor-parallel all-reduce).

Everything on the PE runs in bf16 (fp32 PSUM accumulation); measured
end-to-end max-rel error vs the fp32 reference is ~4e-3, well inside the
2e-2 gate, and bf16 halves DMA traffic, halves SBUF footprint (so all
weights + both batches' activations stay resident) and unlocks the
2-4x DVE 16-bit modes for the softmax bookkeeping.

Structure (emission order = engine program order):
  P1(b0), P1(b1):  projections + rope, TWO passes per batch over x
      (pass A: q0,q1,k; pass B: q2,q3,v).  3 accumulation groups x
      bufs=2 PSUM banks -> evictions of chunk c overlap the full 20us
      K-sweep of chunk c+1, so the PE never waits on a bank.  x is read
      twice (bf16 makes the 2x stream fit in HBM bandwidth); weights
      are loaded once up front, in k-group tiles so the first matmul
      only waits for ~1.5MB.
  P2(b0), P2(b1):  attention + o-projection per 512-token q-chunk.
      Scores transposed (sT = kT.T @ qT) so AV contracts s on the
      partition dim.  Softmax denominator comes from DVE adds of the
      exp tiles (off the PE) + ONE all-ones [128,128] matmul per
      (chunk, head) that sums over partitions AND broadcasts in one
      shot; 1/l via the fast custom-DVE reciprocal.  q-chunks are
      processed in pairs {3,0},{2,1} with heads interleaved so each
      stream's finalize chain hides behind a long stream's matmuls,
      and the o-projection of finished chunks is emitted between
      streams to keep the PE queue deep.
"""

import os
import sys

sys.path.insert(0, "/opt/trn_rl_repo")

import numpy as np

B = 2
T = 2048
TOK = B * T
D = 4096
NQ = 32
NKV = 8
H = 128
HH = H // 2
THETA = 10000.0
NCORES = 8
NHC = NQ // NCORES          # q heads per core (4)
KPC = D // H                # contraction chunks of 128 over D (32)
KG = 4                      # k-groups per weight tensor (8 chunks each)
TCH = 512                   # token chunk
NTCH = T // TCH             # 4 token chunks per batch
NSUB = TCH // H             # 4 s-subtiles per chunk
C_SM = 1.0 / np.sqrt(H)     # softmax scale


def _build_bass():
    import concourse.bacc as bacc
    import concourse.mybir as mybir
    import concourse.tile as tile
    from concourse.masks import make_identity
    from contextlib import ExitStack

    f32 = mybir.dt.float32
    bf16 = mybir.dt.bfloat16
    Exp = mybir.ActivationFunctionType.Exp
    Copy = mybir.ActivationFunctionType.Copy

    nc = bacc.Bacc("TRN2", target_bir_lowering=False, debug=False,
                   num_devices=NCORES)

    xT = nc.declare_dram_parameter("xT", [D, TOK], bf16, isOutput=False)
    # host pre-shuffled so every DMA row is >=2KB contiguous:
    # wqs[p, h, c, m] = wq[h, c*128+p, m]
    wqs = nc.declare_dram_parameter("wqs", [H, NHC, KPC, H], bf16,
                                    isOutput=False)
    wks = nc.declare_dram_parameter("wks", [H, KPC, H], bf16, isOutput=False)
    wvs = nc.declare_dram_parameter("wvs", [H, KPC, H], bf16, isOutput=False)
    # wos[p, h, d] = wo[h, p, d]
    wos = nc.declare_dram_parameter("wos", [H, NHC, D], bf16, isOutput=False)
    # rope tables duplicated across partition halves; sinT's TOP half is
    # NEGATED on the host so rope is out = direct*cosT + swap*sinT for all
    # 128 partitions in one mul+mul+add.
    cosT = nc.declare_dram_parameter("cosT", [H, TOK], bf16, isOutput=False)
    sinT = nc.declare_dram_parameter("sinT", [H, TOK], bf16, isOutput=False)
    o_part = nc.declare_dram_parameter("o_part", [TOK, D], bf16, isOutput=True)
    # x viewed as [p, kchunk, t] so one DMA start can fetch 4 k-chunks
    # (each dma_start costs ~600ns of serial Sync-sequencer time; the
    # un-batched version saturated that queue)
    xTv = xT.rearrange("(c p) t -> p c t", p=H)

    with tile.TileContext(nc) as tc:
        with ExitStack() as top:
            consts = top.enter_context(tc.tile_pool(name="consts", bufs=1))
            identity = consts.tile([H, H], bf16)
            make_identity(nc, identity)
            ones128 = consts.tile([H, H], bf16, tag="ones128")
            nc.vector.memset(ones128, 1.0)
            # 0/1 causal wedge for the 128x128 block that straddles the
            # diagonal: wedge[s, t'] = 1 iff t' >= s.  Blocks left of it are
            # skipped entirely (matmuls narrowed), blocks right of it are
            # all-ones (no mask needed).
            wedge = consts.tile([H, H], bf16, tag="wedge")
            nc.vector.memset(wedge, 1.0)
            nc.gpsimd.affine_select(
                out=wedge, in_=wedge,
                compare_op=mybir.AluOpType.is_ge,
                fill=0.0,
                base=0,
                pattern=[[1, H]],
                channel_multiplier=-1,
            )

            # ---- weights: loaded once, staged so x streaming stays ahead ----
            wpool = top.enter_context(tc.tile_pool(name="wpool", bufs=1))
            wq_t = [[wpool.tile([H, 8, H], bf16, tag=f"wq{h}_{g}",
                                name=f"wq{h}_{g}") for g in range(KG)]
                    for h in range(NHC)]
            wk_t = [wpool.tile([H, 8, H], bf16, tag=f"wk{g}", name=f"wk{g}")
                    for g in range(KG)]
            wv_t = [wpool.tile([H, 8, H], bf16, tag=f"wv{g}", name=f"wv{g}")
                    for g in range(KG)]
            wo_t = [wpool.tile([H, NHC, 1024], bf16, tag=f"wo{dq}",
                               name=f"wo{dq}") for dq in range(4)]
            cos_t = [wpool.tile([H, T], bf16, tag=f"cos{b}", name=f"cos{b}")
                     for b in range(B)]
            sin_t = [wpool.tile([H, T], bf16, tag=f"sin{b}", name=f"sin{b}")
                     for b in range(B)]

            # immediately needed: pass-A k-group 0; everything else is
            # drained between x loads so the first x tile isn't queued
            # behind megabytes of weights
            nc.sync.dma_start(out=wq_t[0][0], in_=wqs[:, 0, 0:8, :])
            nc.sync.dma_start(out=wq_t[1][0], in_=wqs[:, 1, 0:8, :])
            nc.sync.dma_start(out=wk_t[0], in_=wks[:, 0:8, :])
            pend = []
            for g in range(1, KG):
                pend.append((wq_t[0][g], wqs[:, 0, g * 8:(g + 1) * 8, :]))
                pend.append((wq_t[1][g], wqs[:, 1, g * 8:(g + 1) * 8, :]))
                pend.append((wk_t[g], wks[:, g * 8:(g + 1) * 8, :]))
            pend.append((cos_t[0], cosT[:, 0:T]))
            pend.append((sin_t[0], sinT[:, 0:T]))
            for g in range(KG):
                pend.append((wq_t[2][g], wqs[:, 2, g * 8:(g + 1) * 8, :]))
                pend.append((wq_t[3][g], wqs[:, 3, g * 8:(g + 1) * 8, :]))
                pend.append((wv_t[g], wvs[:, g * 8:(g + 1) * 8, :]))
            pend.append((cos_t[1], cosT[:, T:TOK]))
            pend.append((sin_t[1], sinT[:, T:TOK]))
            for dq in range(4):
                pend.append((wo_t[dq], wos[:, :, dq * 1024:(dq + 1) * 1024]))

            def drain_pend(n):
                for _ in range(n):
                    if pend:
                        dst, src = pend.pop(0)
                        nc.sync.dma_start(out=dst, in_=src)

            # ---- activations, both batches resident (bf16) ----
            apool = top.enter_context(tc.tile_pool(name="apool", bufs=1))
            qTs = [[apool.tile([H, NHC, TCH], bf16, tag=f"qT{b}_{i}",
                               name=f"qT{b}_{i}") for i in range(NTCH)]
                   for b in range(B)]
            kTs = [[apool.tile([H, TCH], bf16, tag=f"kT{b}_{i}",
                               name=f"kT{b}_{i}") for i in range(NTCH)]
                   for b in range(B)]
            vs = [[apool.tile([H, NSUB, H], bf16, tag=f"v{b}_{i}",
                              name=f"v{b}_{i}") for i in range(NTCH)]
                  for b in range(B)]

            # ================= phase 1: projections + rope =================
            with ExitStack() as ph1:
                xpool = ph1.enter_context(tc.tile_pool(name="xpool", bufs=5))
                rtmp = ph1.enter_context(tc.tile_pool(name="rtmp", bufs=2))
                pj = ph1.enter_context(
                    tc.tile_pool(name="pj", bufs=2, space="PSUM"))
                pt = ph1.enter_context(
                    tc.tile_pool(name="pt", bufs=2, space="PSUM"))

                def rope_from_psum(psum, dst_ap, cs, sn):
                    # swap staging: halves exchanged so the mul against the
                    # (half-duplicated) rope table is one full-width op.
                    swap = rtmp.tile([H, TCH], f32, tag="swap", bufs=3,
                                     name="swap")
                    nc.vector.tensor_copy(swap[0:HH, :], psum[HH:H, :])
                    nc.vector.tensor_copy(swap[HH:H, :], psum[0:HH, :])
                    m1 = rtmp.tile([H, TCH], f32, tag="m1", name="m1")
                    m2 = rtmp.tile([H, TCH], f32, tag="m2", name="m2")
                    nc.vector.tensor_mul(m1, psum, cs)
                    nc.vector.tensor_mul(m2, swap, sn)
                    nc.vector.tensor_add(dst_ap, m1, m2)

                # staged weight-DMA drain counts, interleaved between the
                # x loads of batch 0 (emission precedes every consumer —
                # Tile deps follow emission order; kg g's weights are
                # drained right before the x group that consumes them).
                drains = {0: [[0, 3, 3, 5], [0, 4, 0, 0], [0, 4, 0, 0],
                              [0, 4, 0, 0]],
                          1: [[0, 2, 0, 0], [0, 2, 0, 0], [0, 2, 0, 0],
                              [0, 0, 0, 0]]}
                for b in range(B):
                    tb = b * T
                    for pas in range(2):
                        for tch in range(NTCH):
                            t0 = tch * TCH
                            g_ps = [pj.tile([H, TCH], f32, tag=f"g{i}",
                                            name=f"g{i}") for i in range(3)]
                            for kq in range(KPC // 8):
                                if b == 0:
                                    drain_pend(drains[pas][tch][kq])
                                x_t = xpool.tile([H, 8, TCH], bf16, tag="x",
                                                 name="x_t")
                                nc.sync.dma_start(
                                    out=x_t,
                                    in_=xTv[:, kq * 8:(kq + 1) * 8,
                                            tb + t0:tb + t0 + TCH])
                                for kc in range(8):
                                    k = kq * 8 + kc
                                    if pas == 0:
                                        lhs = [wq_t[0][kq][:, kc, :],
                                               wq_t[1][kq][:, kc, :],
                                               wk_t[kq][:, kc, :]]
                                    else:
                                        lhs = [wq_t[2][kq][:, kc, :],
                                               wq_t[3][kq][:, kc, :],
                                               wv_t[kq][:, kc, :]]
                                    for gi in range(3):
                                        nc.tensor.matmul(
                                            g_ps[gi], lhs[gi], x_t[:, kc, :],
                                            start=(k == 0),
                                            stop=(k == KPC - 1),
                                            skip_group_check=True)
                            cs = cos_t[b][:, t0:t0 + TCH]
                            sn = sin_t[b][:, t0:t0 + TCH]
                            if pas == 0:
                                rope_from_psum(g_ps[2], kTs[b][tch], cs, sn)
                                rope_from_psum(g_ps[0], qTs[b][tch][:, 0, :],
                                               cs, sn)
                                rope_from_psum(g_ps[1], qTs[b][tch][:, 1, :],
                                               cs, sn)
                            else:
                                vstage = rtmp.tile([H, TCH], bf16,
                                                   tag="vstage", name="vstage")
                                nc.vector.tensor_copy(vstage, g_ps[2])
                                for j in range(NSUB):
                                    tp = pt.tile([H, H], bf16, tag="vtp",
                                                 name="vtp")
                                    nc.tensor.transpose(
                                        tp, vstage[:, j * H:(j + 1) * H],
                                        identity)
                                    nc.vector.tensor_copy(
                                        vs[b][tch][:, j, :], tp)
                                rope_from_psum(g_ps[0], qTs[b][tch][:, 2, :],
                                               cs, sn)
                                rope_from_psum(g_ps[1], qTs[b][tch][:, 3, :],
                                               cs, sn)

            # ============= phase 2: attention + o-projection =============
            with ExitStack() as ph2:
                p2pool = ph2.enter_context(tc.tile_pool(name="p2pool", bufs=4))
                lpool = ph2.enter_context(tc.tile_pool(name="lpool", bufs=2))
                rpool = ph2.enter_context(tc.tile_pool(name="rpool", bufs=2))
                otpool = ph2.enter_context(tc.tile_pool(name="otpool", bufs=1))
                opool = ph2.enter_context(tc.tile_pool(name="opool", bufs=2))
                ps_s = ph2.enter_context(
                    tc.tile_pool(name="ps_s", bufs=3, space="PSUM"))
                ps_av = ph2.enter_context(
                    tc.tile_pool(name="ps_av", bufs=2, space="PSUM"))
                ps_lbc = ph2.enter_context(
                    tc.tile_pool(name="ps_lbc", bufs=1, space="PSUM"))
                ps_o = ph2.enter_context(
                    tc.tile_pool(name="ps_o", bufs=2, space="PSUM"))

                def attn_stream(b, outTs, qc, h, fin_prev):
                    """Emit one (q-chunk, head) stream's matmuls; return a
                    finalize thunk (denominator broadcast + reciprocal +
                    normalize) that the NEXT stream runs after its first
                    score block, so the PE never waits on the DVE lsum
                    tail."""
                    n_st = (qc + 1) * NSUB
                    rhs_q = qTs[b][qc][:, h, :]
                    av_ps = ps_av.tile([H, TCH], f32, tag="av",
                                       name="av_ps")
                    lsum = lpool.tile([H, TCH], bf16, tag="lsum",
                                      name="lsum")

                    def scores_block(st):
                        # diagonal-band tiles are narrowed to the causally
                        # reachable columns t >= j*128; only the 128-wide
                        # block straddling the diagonal needs masking
                        j = st - qc * NSUB
                        nw = j * H if j > 0 else 0
                        sps = ps_s.tile([H, TCH], f32, tag="s", name="sps")
                        kt = kTs[b][st // NSUB][
                            :, (st % NSUB) * H:(st % NSUB + 1) * H]
                        nc.tensor.matmul(sps[:, nw:], kt, rhs_q[:, nw:],
                                         start=True, stop=True)
                        pT2 = p2pool.tile([H, TCH], bf16, tag="p2",
                                          name="pT2")
                        nc.scalar.activation(pT2[:, nw:], sps[:, nw:], Exp,
                                             scale=C_SM)
                        if j >= 0:
                            nc.vector.tensor_mul(pT2[:, nw:nw + H],
                                                 pT2[:, nw:nw + H], wedge)
                        # softmax denominator accumulates on DVE, off the
                        # PE's critical path
                        if st == 0:
                            nc.vector.tensor_copy(lsum, pT2)
                        else:
                            nc.vector.tensor_add(lsum[:, nw:], lsum[:, nw:],
                                                 pT2[:, nw:])
                        return pT2, nw

                    def av_block(st, pT2, nw):
                        nc.tensor.matmul(
                            av_ps[:, nw:],
                            vs[b][st // NSUB][:, st % NSUB, :], pT2[:, nw:],
                            start=(st == 0), stop=(st == n_st - 1),
                            skip_group_check=True)

                    # lookahead-2: two score blocks in flight ahead of each
                    # AV so the exp/mask latency never stalls the PE
                    pending = [scores_block(0)]
                    if fin_prev is not None:
                        fin_prev()
                    pending.append(scores_block(1))
                    for st in range(2, n_st):
                        pending.append(scores_block(st))
                        av_block(st - 2, *pending.pop(0))
                    av_block(n_st - 2, *pending.pop(0))
                    av_block(n_st - 1, *pending.pop(0))

                    def fin():
                        # partition-sum + broadcast of the denominator in
                        # one all-ones matmul, then fast reciprocal +
                        # normalize
                        lbc = ps_lbc.tile([H, TCH], f32, tag="lbc",
                                          name="lbc")
                        nc.tensor.matmul(lbc, ones128, lsum,
                                         start=True, stop=True)
                        rl = rpool.tile([H, TCH], f32, tag="rl", name="rl")
                        nc.vector.reciprocal_approx_fast(out=rl, in_=lbc)
                        nc.vector.tensor_mul(outTs[qc][:, h, :], av_ps, rl)

                    return fin

                def oproj_units(b, outTs, qc):
                    """o-projection of one q-chunk as 8 independent thunks
                    (one per (u, dh)), drained between attention streams to
                    keep the PE queue deep."""
                    tb = b * T
                    outT = outTs[qc]
                    units = []
                    for u in range(NSUB):
                        for dh in range(2):
                            def unit(u=u, dh=dh):
                                trow = tb + qc * TCH + u * H
                                # 4 PSUM evictions batched into one 4KB-row
                                # store: keeps the Sync queue off the
                                # critical path
                                o_sb = opool.tile([H, 4, TCH], bf16,
                                                  tag="osb", name="o_sb")
                                for j in range(4):
                                    dc = dh * 4 + j
                                    ops = ps_o.tile([H, TCH], f32, tag="o",
                                                    name="ops")
                                    for h in range(NHC):
                                        nc.tensor.matmul(
                                            ops,
                                            outT[:, h, u * H:(u + 1) * H],
                                            wo_t[dc // 2][:, h,
                                                          (dc % 2) * TCH:
                                                          (dc % 2 + 1) * TCH],
                                            start=(h == 0),
                                            stop=(h == NHC - 1),
                                            skip_group_check=True)
                                    nc.scalar.activation(
                                        o_sb[:, j, :], ops, Copy)
                                nc.sync.dma_start(
                                    out=o_part[trow:trow + H,
                                               dh * 2048:(dh + 1) * 2048],
                                    in_=o_sb)
                            units.append(unit)
                    return units

                opq = []        # pending o-proj units, carried across batches
                fin = None
                for b in range(B):
                    outTs = {qc: otpool.tile([H, NHC, TCH], bf16,
                                             tag=f"outT{qc}",
                                             name=f"outT{qc}")
                             for qc in range(NTCH)}
                    streams = []
                    for qa, qb in ((3, 0), (2, 1)):
                        for h in range(NHC):
                            streams.append((qa, h))
                            streams.append((qb, h))
                    for i, (qc, h) in enumerate(streams):
                        fin = attn_stream(b, outTs, qc, h, fin)
                        if i == 8:
                            opq += oproj_units(b, outTs, 3)
                        elif i == 9:
                            opq += oproj_units(b, outTs, 0)
                        for _ in range(2):
                            if opq:
                                opq.pop(0)()
                    opq += oproj_units(b, outTs, 2)
                    opq += oproj_units(b, outTs, 1)
                fin()
                for unit in opq:
                    unit()

    nc.compile()
    return nc


_NC_CACHE = None


def _prep_inputs(x, wq, wk, wv, wo, positions):
    import ml_dtypes
    bf = ml_dtypes.bfloat16

    x = np.asarray(x, dtype=np.float32)
    wq = np.asarray(wq, dtype=np.float32)
    wk = np.asarray(wk, dtype=np.float32)
    wv = np.asarray(wv, dtype=np.float32)
    wo = np.asarray(wo, dtype=np.float32)
    positions = np.asarray(positions)

    xT = np.ascontiguousarray(x.reshape(TOK, D).T.astype(bf))
    # rope tables [H, TOK], duplicated across halves, sin top half negated
    fraction = 2.0 * np.arange(HH, dtype=np.float32) / H
    timescale = (THETA ** fraction).astype(np.float32)
    pos = positions.reshape(TOK).astype(np.float32)
    sinu = pos[None, :] / timescale[:, None]
    cos = np.cos(sinu).astype(np.float32)
    sin = np.sin(sinu).astype(np.float32)
    cosT = np.ascontiguousarray(np.concatenate([cos, cos], 0).astype(bf))
    sinT = np.ascontiguousarray(np.concatenate([-sin, sin], 0).astype(bf))

    in_maps = []
    for c in range(NCORES):
        wq_c = wq[c * NHC:(c + 1) * NHC]            # [4, D, H]
        wqs = np.ascontiguousarray(
            wq_c.reshape(NHC, KPC, H, H).transpose(2, 0, 1, 3).astype(bf))
        wks = np.ascontiguousarray(
            wk[c].reshape(KPC, H, H).transpose(1, 0, 2).astype(bf))
        wvs = np.ascontiguousarray(
            wv[c].reshape(KPC, H, H).transpose(1, 0, 2).astype(bf))
        wos = np.ascontiguousarray(
            wo[c * NHC:(c + 1) * NHC].transpose(1, 0, 2).astype(bf))
        in_maps.append({
            "xT": xT,
            "wqs": wqs,
            "wks": wks,
            "wvs": wvs,
            "wos": wos,
            "cosT": cosT,
            "sinT": sinT,
        })
    return in_maps


def kernel(x, wq, wk, wv, wo, positions):
    global _NC_CACHE
    from concourse.bass_utils import run_bass_kernel_spmd

    in_maps = _prep_inputs(x, wq, wk, wv, wo, positions)

    if _NC_CACHE is None:
        _NC_CACHE = _build_bass()
    nc = _NC_CACHE

    trace = os.environ.get("BASS_KERNEL_TRACE", "0") == "1"
    res = run_bass_kernel_spmd(nc, in_maps, list(range(NCORES)), trace=trace)
    global LAST_RESULTS
    LAST_RESULTS = res
    out = np.zeros((TOK, D), dtype=np.float32)
    for c in range(NCORES):
        out += np.asarray(res.results[c]["o_part"]).astype(np.float32)
    return out.reshape(B, T, D)


LAST_RESULTS = None


# revision 24
# speedup vs baseline: 1.6767x; 1.0053x over previous
"""GQA causal-attention prefill kernel for Trainium2, tensor-parallel over 8 NeuronCores.

Reference semantics: q/k/v projections + RoPE + causal GQA attention +
output projection, B=2, T=2048, D=4096, 32 q heads, 8 kv heads, head_dim
128.  Core c owns q heads [4c, 4c+4), kv head c and the matching wo
slice; each core computes a full-shape partial output o_part and the
host sums the 8 partials (the tensor-parallel all-reduce).

Everything on the PE runs in bf16 (fp32 PSUM accumulation); measured
end-to-end max-rel error vs the fp32 reference is ~4e-3, well inside the
2e-2 gate, and bf16 halves DMA traffic, halves SBUF footprint (so all
weights + both batches' activations stay resident) and unlocks the
2-4x DVE 16-bit modes for the softmax bookkeeping.

Structure (emission order = engine program order):
  P1(b0), P1(b1):  projections + rope, TWO passes per batch over x
      (pass A: q0,q1,k; pass B: q2,q3,v).  3 accumulation groups x
      bufs=2 PSUM banks -> evictions of chunk c overlap the full 20us
      K-sweep of chunk c+1, so the PE never waits on a bank.  x is read
      twice (bf16 makes the 2x stream fit in HBM bandwidth); weights
      are loaded once up front, in k-group tiles so the first matmul
      only waits for ~1.5MB.
  P2(b0), P2(b1):  attention + o-projection per 512-token q-chunk.
      Scores transposed (sT = kT.T @ qT) so AV contracts s on the
      partition dim.  Softmax denominator comes from DVE adds of the
      exp tiles (off the PE) + ONE all-ones [128,128] matmul per
      (chunk, head) that sums over partitions AND broadcasts in one
      shot; 1/l via the fast custom-DVE reciprocal.  q-chunks are
      processed in pairs {3,0},{2,1} with heads interleaved so each
      stream's finalize chain hides behind a long stream's matmuls,
      and the o-projection of finished chunks is emitted between
      streams to keep the PE queue deep.
"""

import os
import sys

sys.path.insert(0, "/opt/trn_rl_repo")

import numpy as np

B = 2
T = 2048
TOK = B * T
D = 4096
NQ = 32
NKV = 8
H = 128
HH = H // 2
THETA = 10000.0
NCORES = 8
NHC = NQ // NCORES          # q heads per core (4)
KPC = D // H                # contraction chunks of 128 over D (32)
KG = 4                      # k-groups per weight tensor (8 chunks each)
TCH = 512                   # token chunk
NTCH = T // TCH             # 4 token chunks per batch
NSUB = TCH // H             # 4 s-subtiles per chunk
C_SM = 1.0 / np.sqrt(H)     # softmax scale


def _build_bass():
    import concourse.bacc as bacc
    import concourse.mybir as mybir
    import concourse.tile as tile
    from concourse.masks import make_identity
    from contextlib import ExitStack

    f32 = mybir.dt.float32
    bf16 = mybir.dt.bfloat16
    Exp = mybir.ActivationFunctionType.Exp
    Copy = mybir.ActivationFunctionType.Copy

    nc = bacc.Bacc("TRN2", target_bir_lowering=False, debug=False,
                   num_devices=NCORES)

    xT = nc.declare_dram_parameter("xT", [D, TOK], bf16, isOutput=False)
    # host pre-shuffled so every DMA row is >=2KB contiguous:
    # wqs[p, h, c, m] = wq[h, c*128+p, m]
    wqs = nc.declare_dram_parameter("wqs", [H, NHC, KPC, H], bf16,
                                    isOutput=False)
    wks = nc.declare_dram_parameter("wks", [H, KPC, H], bf16, isOutput=False)
    wvs = nc.declare_dram_parameter("wvs", [H, KPC, H], bf16, isOutput=False)
    # wos[p, h, d] = wo[h, p, d]
    wos = nc.declare_dram_parameter("wos", [H, NHC, D], bf16, isOutput=False)
    # rope tables duplicated across partition halves; sinT's TOP half is
    # NEGATED on the host so rope is out = direct*cosT + swap*sinT for all
    # 128 partitions in one mul+mul+add.
    cosT = nc.declare_dram_parameter("cosT", [H, TOK], bf16, isOutput=False)
    sinT = nc.declare_dram_parameter("sinT", [H, TOK], bf16, isOutput=False)
    o_part = nc.declare_dram_parameter("o_part", [TOK, D], bf16, isOutput=True)
    # x viewed as [p, kchunk, t] so one DMA start can fetch 4 k-chunks
    # (each dma_start costs ~600ns of serial Sync-sequencer time; the
    # un-batched version saturated that queue)
    xTv = xT.rearrange("(c p) t -> p c t", p=H)

    with tile.TileContext(nc) as tc:
        with ExitStack() as top:
            consts = top.enter_context(tc.tile_pool(name="consts", bufs=1))
            identity = consts.tile([H, H], bf16)
            make_identity(nc, identity)
            ones128 = consts.tile([H, H], bf16, tag="ones128")
            nc.vector.memset(ones128, 1.0)
            # 0/1 causal wedge for the 128x128 block that straddles the
            # diagonal: wedge[s, t'] = 1 iff t' >= s.  Blocks left of it are
            # skipped entirely (matmuls narrowed), blocks right of it are
            # all-ones (no mask needed).
            wedge = consts.tile([H, H], bf16, tag="wedge")
            nc.vector.memset(wedge, 1.0)
            nc.gpsimd.affine_select(
                out=wedge, in_=wedge,
                compare_op=mybir.AluOpType.is_ge,
                fill=0.0,
                base=0,
                pattern=[[1, H]],
                channel_multiplier=-1,
            )

            # ---- weights: loaded once, staged so x streaming stays ahead ----
            wpool = top.enter_context(tc.tile_pool(name="wpool", bufs=1))
            wq_t = [[wpool.tile([H, 8, H], bf16, tag=f"wq{h}_{g}",
                                name=f"wq{h}_{g}") for g in range(KG)]
                    for h in range(NHC)]
            wk_t = [wpool.tile([H, 8, H], bf16, tag=f"wk{g}", name=f"wk{g}")
                    for g in range(KG)]
            wv_t = [wpool.tile([H, 8, H], bf16, tag=f"wv{g}", name=f"wv{g}")
                    for g in range(KG)]
            wo_t = [wpool.tile([H, NHC, 1024], bf16, tag=f"wo{dq}",
                               name=f"wo{dq}") for dq in range(4)]
            cos_t = [wpool.tile([H, T], bf16, tag=f"cos{b}", name=f"cos{b}")
                     for b in range(B)]
            sin_t = [wpool.tile([H, T], bf16, tag=f"sin{b}", name=f"sin{b}")
                     for b in range(B)]

            # immediately needed: pass-A k-group 0; everything else is
            # drained between x loads so the first x tile isn't queued
            # behind megabytes of weights
            nc.sync.dma_start(out=wq_t[0][0], in_=wqs[:, 0, 0:8, :])
            nc.sync.dma_start(out=wq_t[1][0], in_=wqs[:, 1, 0:8, :])
            nc.sync.dma_start(out=wk_t[0], in_=wks[:, 0:8, :])
            pend = []
            for g in range(1, KG):
                pend.append((wq_t[0][g], wqs[:, 0, g * 8:(g + 1) * 8, :]))
                pend.append((wq_t[1][g], wqs[:, 1, g * 8:(g + 1) * 8, :]))
                pend.append((wk_t[g], wks[:, g * 8:(g + 1) * 8, :]))
            pend.append((cos_t[0], cosT[:, 0:T]))
            pend.append((sin_t[0], sinT[:, 0:T]))
            for g in range(KG):
                pend.append((wq_t[2][g], wqs[:, 2, g * 8:(g + 1) * 8, :]))
                pend.append((wq_t[3][g], wqs[:, 3, g * 8:(g + 1) * 8, :]))
                pend.append((wv_t[g], wvs[:, g * 8:(g + 1) * 8, :]))
            pend.append((cos_t[1], cosT[:, T:TOK]))
            pend.append((sin_t[1], sinT[:, T:TOK]))
            for dq in range(4):
                pend.append((wo_t[dq], wos[:, :, dq * 1024:(dq + 1) * 1024]))

            def drain_pend(n):
                for _ in range(n):
                    if pend:
                        dst, src = pend.pop(0)
                        nc.sync.dma_start(out=dst, in_=src)

            # ---- activations, both batches resident (bf16) ----
            apool = top.enter_context(tc.tile_pool(name="apool", bufs=1))
            qTs = [[apool.tile([H, NHC, TCH], bf16, tag=f"qT{b}_{i}",
                               name=f"qT{b}_{i}") for i in range(NTCH)]
                   for b in range(B)]
            kTs = [[apool.tile([H, TCH], bf16, tag=f"kT{b}_{i}",
                               name=f"kT{b}_{i}") for i in range(NTCH)]
                   for b in range(B)]
            vs = [[apool.tile([H, NSUB, H], bf16, tag=f"v{b}_{i}",
                              name=f"v{b}_{i}") for i in range(NTCH)]
                  for b in range(B)]

            # ================= phase 1: projections + rope =================
            with ExitStack() as ph1:
                xpool = ph1.enter_context(tc.tile_pool(name="xpool", bufs=5))
                rtmp = ph1.enter_context(tc.tile_pool(name="rtmp", bufs=2))
                pj = ph1.enter_context(
                    tc.tile_pool(name="pj", bufs=2, space="PSUM"))
                pt = ph1.enter_context(
                    tc.tile_pool(name="pt", bufs=2, space="PSUM"))

                def rope_from_psum(psum, dst_ap, cs, sn):
                    # swap staging: halves exchanged so the mul against the
                    # (half-duplicated) rope table is one full-width op.
                    swap = rtmp.tile([H, TCH], f32, tag="swap", bufs=3,
                                     name="swap")
                    nc.vector.tensor_copy(swap[0:HH, :], psum[HH:H, :])
                    nc.vector.tensor_copy(swap[HH:H, :], psum[0:HH, :])
                    m1 = rtmp.tile([H, TCH], f32, tag="m1", name="m1")
                    m2 = rtmp.tile([H, TCH], f32, tag="m2", name="m2")
                    nc.vector.tensor_mul(m1, psum, cs)
                    nc.vector.tensor_mul(m2, swap, sn)
                    nc.vector.tensor_add(dst_ap, m1, m2)

                # staged weight-DMA drain counts, interleaved between the
                # x loads of batch 0 (emission precedes every consumer —
                # Tile deps follow emission order; kg g's weights are
                # drained right before the x group that consumes them).
                drains = {0: [[0, 3, 3, 5], [0, 4, 0, 0], [0, 4, 0, 0],
                              [0, 4, 0, 0]],
                          1: [[0, 2, 0, 0], [0, 2, 0, 0], [0, 2, 0, 0],
                              [0, 0, 0, 0]]}
                for b in range(B):
                    tb = b * T
                    for pas in range(2):
                        for tch in range(NTCH):
                            t0 = tch * TCH
                            g_ps = [pj.tile([H, TCH], f32, tag=f"g{i}",
                                            name=f"g{i}") for i in range(3)]
                            for kq in range(KPC // 8):
                                if b == 0:
                                    drain_pend(drains[pas][tch][kq])
                                x_t = xpool.tile([H, 8, TCH], bf16, tag="x",
                                                 name="x_t")
                                nc.sync.dma_start(
                                    out=x_t,
                                    in_=xTv[:, kq * 8:(kq + 1) * 8,
                                            tb + t0:tb + t0 + TCH])
                                for kc in range(8):
                                    k = kq * 8 + kc
                                    if pas == 0:
                                        lhs = [wq_t[0][kq][:, kc, :],
                                               wq_t[1][kq][:, kc, :],
                                               wk_t[kq][:, kc, :]]
                                    else:
                                        lhs = [wq_t[2][kq][:, kc, :],
                                               wq_t[3][kq][:, kc, :],
                                               wv_t[kq][:, kc, :]]
                                    for gi in range(3):
                                        nc.tensor.matmul(
                                            g_ps[gi], lhs[gi], x_t[:, kc, :],
                                            start=(k == 0),
                                            stop=(k == KPC - 1),
                                            skip_group_check=True)
                            cs = cos_t[b][:, t0:t0 + TCH]
                            sn = sin_t[b][:, t0:t0 + TCH]
                            if pas == 0:
                                rope_from_psum(g_ps[2], kTs[b][tch], cs, sn)
                                rope_from_psum(g_ps[0], qTs[b][tch][:, 0, :],
                                               cs, sn)
                                rope_from_psum(g_ps[1], qTs[b][tch][:, 1, :],
                                               cs, sn)
                            else:
                                # q-ropes first: their PSUM banks free
                                # sooner, which is what phase 2's first
                                # scores wait on at the P1->P2 boundary
                                rope_from_psum(g_ps[0], qTs[b][tch][:, 2, :],
                                               cs, sn)
                                rope_from_psum(g_ps[1], qTs[b][tch][:, 3, :],
                                               cs, sn)
                                vstage = rtmp.tile([H, TCH], bf16,
                                                   tag="vstage", name="vstage")
                                nc.vector.tensor_copy(vstage, g_ps[2])
                                for j in range(NSUB):
                                    tp = pt.tile([H, H], bf16, tag="vtp",
                                                 name="vtp")
                                    nc.tensor.transpose(
                                        tp, vstage[:, j * H:(j + 1) * H],
                                        identity)
                                    nc.vector.tensor_copy(
                                        vs[b][tch][:, j, :], tp)

            # ============= phase 2: attention + o-projection =============
            with ExitStack() as ph2:
                p2pool = ph2.enter_context(tc.tile_pool(name="p2pool", bufs=4))
                lpool = ph2.enter_context(tc.tile_pool(name="lpool", bufs=2))
                rpool = ph2.enter_context(tc.tile_pool(name="rpool", bufs=2))
                otpool = ph2.enter_context(tc.tile_pool(name="otpool", bufs=1))
                opool = ph2.enter_context(tc.tile_pool(name="opool", bufs=2))
                ps_s = ph2.enter_context(
                    tc.tile_pool(name="ps_s", bufs=3, space="PSUM"))
                ps_av = ph2.enter_context(
                    tc.tile_pool(name="ps_av", bufs=2, space="PSUM"))
                ps_lbc = ph2.enter_context(
                    tc.tile_pool(name="ps_lbc", bufs=1, space="PSUM"))
                ps_o = ph2.enter_context(
                    tc.tile_pool(name="ps_o", bufs=2, space="PSUM"))

                def attn_stream(b, outTs, qc, h):
                    """Emit one (q-chunk, head) stream: scores/AV matmuls
                    plus finalize (denominator broadcast + reciprocal +
                    normalize).  The final lsum add is a narrow diagonal
                    tile, so the broadcast matmul never waits on DVE."""
                    n_st = (qc + 1) * NSUB
                    rhs_q = qTs[b][qc][:, h, :]
                    av_ps = ps_av.tile([H, TCH], f32, tag="av",
                                       name="av_ps")
                    lsum = lpool.tile([H, TCH], bf16, tag="lsum",
                                      name="lsum")

                    def scores_block(st):
                        # diagonal-band tiles are narrowed to the causally
                        # reachable columns t >= j*128; only the 128-wide
                        # block straddling the diagonal needs masking
                        j = st - qc * NSUB
                        nw = j * H if j > 0 else 0
                        sps = ps_s.tile([H, TCH], f32, tag="s", name="sps")
                        kt = kTs[b][st // NSUB][
                            :, (st % NSUB) * H:(st % NSUB + 1) * H]
                        nc.tensor.matmul(sps[:, nw:], kt, rhs_q[:, nw:],
                                         start=True, stop=True)
                        pT2 = p2pool.tile([H, TCH], bf16, tag="p2",
                                          name="pT2")
                        nc.scalar.activation(pT2[:, nw:], sps[:, nw:], Exp,
                                             scale=C_SM)
                        if j >= 0:
                            nc.vector.tensor_mul(pT2[:, nw:nw + H],
                                                 pT2[:, nw:nw + H], wedge)
                        # softmax denominator accumulates on DVE, off the
                        # PE's critical path
                        if st == 0:
                            nc.vector.tensor_copy(lsum, pT2)
                        else:
                            nc.vector.tensor_add(lsum[:, nw:], lsum[:, nw:],
                                                 pT2[:, nw:])
                        return pT2, nw

                    def av_block(st, pT2, nw):
                        nc.tensor.matmul(
                            av_ps[:, nw:],
                            vs[b][st // NSUB][:, st % NSUB, :], pT2[:, nw:],
                            start=(st == 0), stop=(st == n_st - 1),
                            skip_group_check=True)

                    # lookahead-2: two score blocks in flight ahead of each
                    # AV so the exp/mask latency never stalls the PE
                    pending = [scores_block(0), scores_block(1)]
                    for st in range(2, n_st):
                        pending.append(scores_block(st))
                        av_block(st - 2, *pending.pop(0))
                    av_block(n_st - 2, *pending.pop(0))
                    av_block(n_st - 1, *pending.pop(0))
                    # partition-sum + broadcast of the denominator in one
                    # all-ones matmul, then fast reciprocal + normalize
                    lbc = ps_lbc.tile([H, TCH], f32, tag="lbc", name="lbc")
                    nc.tensor.matmul(lbc, ones128, lsum,
                                     start=True, stop=True)
                    rl = rpool.tile([H, TCH], f32, tag="rl", name="rl")
                    nc.vector.reciprocal_approx_fast(out=rl, in_=lbc)
                    nc.vector.tensor_mul(outTs[qc][:, h, :], av_ps, rl)

                def oproj_units(b, outTs, qc):
                    """o-projection of one q-chunk as 8 independent thunks
                    (one per (u, dh)), drained between attention streams to
                    keep the PE queue deep."""
                    tb = b * T
                    outT = outTs[qc]
                    units = []
                    for u in range(NSUB):
                        for dh in range(2):
                            def unit(u=u, dh=dh):
                                trow = tb + qc * TCH + u * H
                                # 4 PSUM evictions batched into one 4KB-row
                                # store: keeps the Sync queue off the
                                # critical path
                                o_sb = opool.tile([H, 4, TCH], bf16,
                                                  tag="osb", name="o_sb")
                                for j in range(4):
                                    dc = dh * 4 + j
                                    ops = ps_o.tile([H, TCH], f32, tag="o",
                                                    name="ops")
                                    for h in range(NHC):
                                        nc.tensor.matmul(
                                            ops,
                                            outT[:, h, u * H:(u + 1) * H],
                                            wo_t[dc // 2][:, h,
                                                          (dc % 2) * TCH:
                                                          (dc % 2 + 1) * TCH],
                                            start=(h == 0),
                                            stop=(h == NHC - 1),
                                            skip_group_check=True)
                                    nc.scalar.activation(
                                        o_sb[:, j, :], ops, Copy)
                                nc.sync.dma_start(
                                    out=o_part[trow:trow + H,
                                               dh * 2048:(dh + 1) * 2048],
                                    in_=o_sb)
                            units.append(unit)
                    return units

                opq = []        # pending o-proj units, carried across batches
                for b in range(B):
                    outTs = {qc: otpool.tile([H, NHC, TCH], bf16,
                                             tag=f"outT{qc}",
                                             name=f"outT{qc}")
                             for qc in range(NTCH)}
                    streams = []
                    for qa, qb in ((3, 0), (2, 1)):
                        for h in range(NHC):
                            streams.append((qa, h))
                            streams.append((qb, h))
                    for i, (qc, h) in enumerate(streams):
                        attn_stream(b, outTs, qc, h)
                        if i == 8:
                            opq += oproj_units(b, outTs, 3)
                        elif i == 9:
                            opq += oproj_units(b, outTs, 0)
                        for _ in range(2):
                            if opq:
                                opq.pop(0)()
                    opq += oproj_units(b, outTs, 2)
                    opq += oproj_units(b, outTs, 1)
                for unit in opq:
                    unit()

    nc.compile()
    return nc


_NC_CACHE = None


def _prep_inputs(x, wq, wk, wv, wo, positions):
    import ml_dtypes
    bf = ml_dtypes.bfloat16

    x = np.asarray(x, dtype=np.float32)
    wq = np.asarray(wq, dtype=np.float32)
    wk = np.asarray(wk, dtype=np.float32)
    wv = np.asarray(wv, dtype=np.float32)
    wo = np.asarray(wo, dtype=np.float32)
    positions = np.asarray(positions)

    xT = np.ascontiguousarray(x.reshape(TOK, D).T.astype(bf))
    # rope tables [H, TOK], duplicated across halves, sin top half negated
    fraction = 2.0 * np.arange(HH, dtype=np.float32) / H
    timescale = (THETA ** fraction).astype(np.float32)
    pos = positions.reshape(TOK).astype(np.float32)
    sinu = pos[None, :] / timescale[:, None]
    cos = np.cos(sinu).astype(np.float32)
    sin = np.sin(sinu).astype(np.float32)
    cosT = np.ascontiguousarray(np.concatenate([cos, cos], 0).astype(bf))
    sinT = np.ascontiguousarray(np.concatenate([-sin, sin], 0).astype(bf))

    in_maps = []
    for c in range(NCORES):
        wq_c = wq[c * NHC:(c + 1) * NHC]            # [4, D, H]
        wqs = np.ascontiguousarray(
            wq_c.reshape(NHC, KPC, H, H).transpose(2, 0, 1, 3).astype(bf))
        wks = np.ascontiguousarray(
            wk[c].reshape(KPC, H, H).transpose(1, 0, 2).astype(bf))
        wvs = np.ascontiguousarray(
            wv[c].reshape(KPC, H, H).transpose(1, 0, 2).astype(bf))
        wos = np.ascontiguousarray(
            wo[c * NHC:(c + 1) * NHC].transpose(1, 0, 2).astype(bf))
        in_maps.append({
            "xT": xT,
            "wqs": wqs,
            "wks": wks,
            "wvs": wvs,
            "wos": wos,
            "cosT": cosT,
            "sinT": sinT,
        })
    return in_maps


def kernel(x, wq, wk, wv, wo, positions):
    global _NC_CACHE
    from concourse.bass_utils import run_bass_kernel_spmd

    in_maps = _prep_inputs(x, wq, wk, wv, wo, positions)

    if _NC_CACHE is None:
        _NC_CACHE = _build_bass()
    nc = _NC_CACHE

    trace = os.environ.get("BASS_KERNEL_TRACE", "0") == "1"
    res = run_bass_kernel_spmd(nc, in_maps, list(range(NCORES)), trace=trace)
    global LAST_RESULTS
    LAST_RESULTS = res
    out = np.zeros((TOK, D), dtype=np.float32)
    for c in range(NCORES):
        out += np.asarray(res.results[c]["o_part"]).astype(np.float32)
    return out.reshape(B, T, D)


LAST_RESULTS = None
